# revision 1
# baseline (speedup 1.0000x reference)
"""MoE (top-2 of 8 experts + 1 shared expert, SwiGLU FFN) on 8 TRN2 NeuronCores.

Strategy (expert-parallel, per the sharding hint):
  - Host computes the (tiny) gate: softmax top-2 over E=8 for T=8192 tokens,
    and from it the dispatch: per-expert gathered token lists, ordered by
    owning core, plus scatter/gather index maps. This is the "sharding" step;
    >99.9% of FLOPs (the FFNs) run on device.
  - Core e receives the tokens routed to expert e (transposed, [D, C_cap]),
    runs the SwiGLU FFN in two token-half passes, scales rows by the gate
    weight, and scatters rows into per-half AllToAll dispatch buffers laid
    out by destination core. Each half's AllToAll fires as soon as that
    half's outputs are ready, overlapping the remaining compute.
  - Each core also runs the shared expert on its own T/8 token slice
    (overlapped with the second collective).
  - Combine on device: out[t] = shared(t) + contrib0(t) + contrib1(t).
  - Host concatenates the 8 [T/8, D] output slices. No host math beyond the
    gate.

Compute dtype is fp16 (11-bit relative precision, ~5e-4 dot-product rel err
vs the fp32 reference), which runs the TensorEngine at full rate with hidden
weight loads and halves all DMA traffic. PSUM accumulation stays fp32.
"""
import contextlib

import numpy as np

import concourse.bass as bass
import concourse.tile as tile
from concourse import bacc, mybir
from concourse.bass_utils import run_bass_kernel_spmd

# problem shape (hardcoded per contract)
T = 8192
D = 1024
F = 4096
E = 8
TOPK = 2
NCORES = 8
TO = T // NCORES          # tokens owned per core

F32 = mybir.dt.float32
F16 = mybir.dt.float16
I32 = mybir.dt.int32

# default capacities; bumped (with recompile) if routing demands more
C_CAP_DEFAULT = 2304      # max tokens per expert (pad target, mult of 256)
P_CAP_DEFAULT = 256       # max tokens per (expert, owner, token-half) chunk

_nc_cache: dict[tuple, object] = {}


def _chunk_slices(c_len):
    """Moving-dim chunks of <=512, each >=256 so LDWEIGHTS stays hidden."""
    out = []
    pos = 0
    rem = c_len
    while rem > 0:
        if rem > 512:
            w = 512 if rem - 512 >= 256 else 384
        else:
            w = rem
        out.append((pos, w))
        pos += w
        rem -= w
    return out


def _ffn_phase(nc, sbuf, psum, xk_tiles, w1d, w2d, b1t, c_len, y_tiles):
    """Emit one SwiGLU FFN pass over c_len tokens.

    xk_tiles: [k][ci] SBUF tiles [128, cw] (fp16), contraction-major slices.
    w1d: DRAM [2F/128=64, D/128=8, 128, 128] fp16 (host-tiled, lhsT layout)
    w2d: DRAM [F, D] fp16
    b1t: SBUF [128, 64] f32 (column m = bias for 2F-chunk m)
    y_tiles: c_len//128 SBUF tiles [128, D] f32 receiving the FFN output
             (written on slab 0, accumulated on slabs 1..3).
    """
    n_t = c_len // 128
    KD = D // 128                       # 8 contraction chunks for stage 1
    MF = F // 128                       # 32 f-chunks
    N_SLAB = 4
    per_slab = MF // N_SLAB             # 8 f-chunks per slab
    chunks = _chunk_slices(c_len)

    for q in range(N_SLAB):
        g_tiles = []
        for fi in range(per_slab):
            mp = q * per_slab + fi      # a-chunk index; b-chunk = mp + MF
            w1a = sbuf.tile([128, KD, 128], F16, tag="w1a", name="w1a", bufs=6)
            w1b = sbuf.tile([128, KD, 128], F16, tag="w1b", name="w1b", bufs=6)
            for k in range(KD):
                nc.sync.dma_start(out=w1a[:, k, :], in_=w1d[mp, k])
                nc.sync.dma_start(out=w1b[:, k, :], in_=w1d[mp + MF, k])
            g_t = sbuf.tile([128, c_len], F16, tag=f"g{fi}", name=f"g{fi}",
                            bufs=1)
            for ci, (cs, cw) in enumerate(chunks):
                ps_a = psum.tile([128, 512], F32, space="PSUM", tag="ps_a",
                                 name="ps_a", bufs=3)
                ps_b = psum.tile([128, 512], F32, space="PSUM", tag="ps_b",
                                 name="ps_b", bufs=3)
                for k in range(KD):
                    nc.tensor.matmul(out=ps_a[:, :cw], lhsT=w1a[:, k, :],
                                     rhs=xk_tiles[k][ci][:, :cw],
                                     start=(k == 0), stop=(k == KD - 1))
                for k in range(KD):
                    nc.tensor.matmul(out=ps_b[:, :cw], lhsT=w1b[:, k, :],
                                     rhs=xk_tiles[k][ci][:, :cw],
                                     start=(k == 0), stop=(k == KD - 1))
                t_a = sbuf.tile([128, 512], F16, tag="t_a", name="t_a", bufs=3)
                t_b = sbuf.tile([128, 512], F16, tag="t_b", name="t_b", bufs=3)
                nc.scalar.activation(t_a[:, :cw], ps_a[:, :cw],
                                     mybir.ActivationFunctionType.Silu,
                                     bias=b1t[:, mp:mp + 1])
                nc.scalar.activation(t_b[:, :cw], ps_b[:, :cw],
                                     mybir.ActivationFunctionType.Identity,
                                     bias=b1t[:, mp + MF:mp + MF + 1])
                nc.vector.tensor_mul(g_t[:, cs:cs + cw], t_a[:, :cw],
                                     t_b[:, :cw])
            g_tiles.append(g_t)
            if fi == 0:
                # stage-2 weights for this slab (stream during stage-1)
                w2t = sbuf.tile([128, per_slab, D], F16, tag="w2",
                                name="w2", bufs=1)
                for fj in range(per_slab):
                    f2 = q * per_slab + fj
                    nc.sync.dma_start(out=w2t[:, fj, :],
                                      in_=w2d[f2 * 128:(f2 + 1) * 128, :])
        # stage-2 partial: y (+)= g_slab.T @ w2_slab
        for t in range(n_t):
            ts = slice(t * 128, (t + 1) * 128)
            for d in range(D // 512):
                ds = slice(d * 512, (d + 1) * 512)
                ps_y = psum.tile([128, 512], F32, space="PSUM", tag="ps_y",
                                 name="ps_y", bufs=2)
                for fi in range(per_slab):
                    nc.tensor.matmul(out=ps_y[:],
                                     lhsT=g_tiles[fi][:, ts],
                                     rhs=w2t[:, fi, ds],
                                     start=(fi == 0), stop=(fi == per_slab - 1))
                if q == 0:
                    nc.vector.tensor_copy(y_tiles[t][:, ds], ps_y[:])
                else:
                    nc.vector.tensor_add(y_tiles[t][:, ds], y_tiles[t][:, ds],
                                         ps_y[:])


def _load_x_chunks(nc, sbuf, xdram, col0, c_len, tag_prefix):
    """Load contraction-major x slices as per-chunk tiles [k][ci]."""
    chunks = _chunk_slices(c_len)
    tiles = [[None] * len(chunks) for _ in range(D // 128)]
    for ci, (cs, cw) in enumerate(chunks):
        for k in range(D // 128):
            xt = sbuf.tile([128, 512], F16, tag=f"{tag_prefix}{k}_{ci}",
                           name=f"xc{k}_{ci}", bufs=2)
            nc.sync.dma_start(out=xt[:, :cw],
                              in_=xdram[k][:, col0 + cs:col0 + cs + cw])
            tiles[k][ci] = xt
    return tiles


def _build(c_cap, p_cap):
    key = (c_cap, p_cap)
    if key in _nc_cache:
        return _nc_cache[key]

    nc = bacc.Bacc("TRN2", target_bir_lowering=False, debug=False,
                   num_devices=NCORES)

    def din(name, shape, dt):
        return nc.dram_tensor(name, shape, dt, kind="ExternalInput").ap()

    xg = din("xg", [D // 128, 128, c_cap], F16)        # gathered tokens^T
    xs = din("xs", [D // 128, 128, TO], F16)           # owned tokens^T
    w1 = din("w1", [2 * F // 128, D // 128, 128, 128], F16)
    w2 = din("w2", [F, D], F16)
    sw1 = din("sw1", [2 * F // 128, D // 128, 128, 128], F16)
    sw2 = din("sw2", [F, D], F16)
    b1 = din("b1", [128, 2 * F // 128], F32)           # col m = chunk-m bias
    sb1 = din("sb1", [128, 2 * F // 128], F32)
    b2 = din("b2", [1, D], F32)
    sb2 = din("sb2", [1, D], F32)
    cwd = din("cw", [c_cap, 1], F32)                   # combine weights
    scat = din("scat", [c_cap, 1], I32)                # row in half's a2a_in
    g0i = din("g0i", [TO, 1], I32)                     # abs row in a2a_out
    g1i = din("g1i", [TO, 1], I32)
    out = nc.dram_tensor("out", [TO, D], F32, kind="ExternalOutput").ap()

    c_half = c_cap // 2
    rows_h = NCORES * p_cap                            # rows per half buffer

    with tile.TileContext(nc) as tc:
        with contextlib.ExitStack() as ctx:
            sbuf = ctx.enter_context(tc.tile_pool(name="sbuf", bufs=1))
            psum = ctx.enter_context(tc.tile_pool(name="psum", bufs=2,
                                                  space="PSUM"))
            dpool = ctx.enter_context(tc.tile_pool(name="dram", bufs=1,
                                                   space="DRAM"))

            a2a_in0 = dpool.tile([rows_h, D], F16)
            a2a_in1 = dpool.tile([rows_h, D], F16)
            a2a_out = dpool.tile([2 * rows_h, D], F16)
            a2a_ins = [a2a_in0, a2a_in1]

            # biases (resident)
            b1t = sbuf.tile([128, 2 * F // 128], F32, tag="b1t", name="b1t",
                            bufs=1)
            sb1t = sbuf.tile([128, 2 * F // 128], F32, tag="sb1t",
                             name="sb1t", bufs=1)
            nc.sync.dma_start(out=b1t[:], in_=b1[:])
            nc.sync.dma_start(out=sb1t[:], in_=sb1[:])
            b2t = sbuf.tile([128, D], F32, tag="b2t", name="b2t", bufs=1)
            sb2t = sbuf.tile([128, D], F32, tag="sb2t", name="sb2t", bufs=1)
            nc.gpsimd.dma_start(out=b2t[:], in_=b2.to_broadcast([128, D]))
            nc.gpsimd.dma_start(out=sb2t[:], in_=sb2.to_broadcast([128, D]))

            # ---------------- routed expert (2 half-passes over tokens) ----
            for h in range(2):
                hs = h * c_half
                xk_tiles = _load_x_chunks(nc, sbuf, xg, hs, c_half, "xk")
                y_tiles = [sbuf.tile([128, D], F32, tag=f"ya{t}",
                                     name=f"ya{t}", bufs=1)
                           for t in range(c_half // 128)]

                _ffn_phase(nc, sbuf, psum, xk_tiles, w1, w2, b1t, c_half,
                           y_tiles)
                # finalize: +b2, *combine weight, scatter to this half's buf
                for t in range(c_half // 128):
                    row0 = hs + t * 128
                    cwt = sbuf.tile([128, 1], F32, tag="cwt", name="cwt",
                                    bufs=2)
                    sct = sbuf.tile([128, 1], I32, tag="sct", name="sct",
                                    bufs=2)
                    nc.sync.dma_start(out=cwt[:], in_=cwd[row0:row0 + 128, :])
                    nc.sync.dma_start(out=sct[:], in_=scat[row0:row0 + 128, :])
                    yt = y_tiles[t]
                    yh = sbuf.tile([128, D], F16, tag="yh", name="yh", bufs=2)
                    nc.vector.tensor_add(yt[:], yt[:], b2t[:])
                    nc.vector.tensor_scalar_mul(yh[:], yt[:], cwt[:, :1])
                    nc.gpsimd.indirect_dma_start(
                        out=a2a_ins[h][:],
                        out_offset=bass.IndirectOffsetOnAxis(ap=sct[:, :1],
                                                             axis=0),
                        in_=yh[:],
                        in_offset=None,
                        bounds_check=rows_h - 1,
                        oob_is_err=False,
                    )
                # dispatch this half back to the owner cores
                nc.gpsimd.collective_compute(
                    "AllToAll",
                    mybir.AluOpType.bypass,
                    replica_groups=[list(range(NCORES))],
                    ins=[a2a_ins[h][:].opt()],
                    outs=[a2a_out[h * rows_h:(h + 1) * rows_h, :].opt()],
                )

            # ---------------- shared expert on owned tokens (overlaps) ----
            sk_tiles = _load_x_chunks(nc, sbuf, xs, 0, TO, "xk")
            ys_tiles = [sbuf.tile([128, D], F32, tag=f"ya{t}", name=f"ya{t}",
                                  bufs=1)
                        for t in range(TO // 128)]

            _ffn_phase(nc, sbuf, psum, sk_tiles, sw1, sw2, sb1t, TO,
                       ys_tiles)

            # combine on GpSimd so it overlaps the tail of the shared phase
            for t in range(TO // 128):
                row0 = t * 128
                i0 = sbuf.tile([128, 1], I32, tag="i0", name="i0", bufs=2)
                i1 = sbuf.tile([128, 1], I32, tag="i1", name="i1", bufs=2)
                nc.sync.dma_start(out=i0[:], in_=g0i[row0:row0 + 128, :])
                nc.sync.dma_start(out=i1[:], in_=g1i[row0:row0 + 128, :])
                r0 = sbuf.tile([128, D], F16, tag="r0", name="r0", bufs=2)
                r1 = sbuf.tile([128, D], F16, tag="r1", name="r1", bufs=2)
                nc.gpsimd.indirect_dma_start(
                    out=r0[:], out_offset=None, in_=a2a_out[:],
                    in_offset=bass.IndirectOffsetOnAxis(ap=i0[:, :1], axis=0))
                nc.gpsimd.indirect_dma_start(
                    out=r1[:], out_offset=None, in_=a2a_out[:],
                    in_offset=bass.IndirectOffsetOnAxis(ap=i1[:, :1], axis=0))
                yt = ys_tiles[t]
                nc.vector.tensor_add(yt[:], yt[:], sb2t[:])
                nc.vector.tensor_add(yt[:], yt[:], r0[:])
                nc.vector.tensor_add(yt[:], yt[:], r1[:])
                nc.sync.dma_start(out=out[row0:row0 + 128, :], in_=yt[:])

    nc.compile()
    _nc_cache[key] = nc
    return nc


def _route(x, gate_w, gate_b):
    """Host gate: softmax top-2 (float64 for stable ordering)."""
    logits = (x.astype(np.float64) @ gate_w.astype(np.float64)
              + gate_b.astype(np.float64))
    m = logits.max(axis=-1, keepdims=True)
    p = np.exp(logits - m)
    p /= p.sum(axis=-1, keepdims=True)
    order = np.argsort(-p, axis=-1)
    idx = order[:, :TOPK]                      # [T, 2]
    wts = np.take_along_axis(p, idx, axis=-1)  # [T, 2]
    return idx, wts.astype(np.float32)


def kernel(x, gate_w, gate_b, shared_w1, shared_b1, shared_w2, shared_b2,
           routed_w1, routed_b1, routed_w2, routed_b2):
    x = np.asarray(x, dtype=np.float32)
    topk_idx, topk_w = _route(x, np.asarray(gate_w), np.asarray(gate_b))

    owner = np.arange(T) // TO                 # owning core per token

    # per-expert dispatch lists, ordered by (owner, token)
    tok_lists, wt_lists = [], []
    for e in range(E):
        sel = (topk_idx == e)                  # [T, 2]
        tsel = np.nonzero(sel.any(axis=1))[0]  # ascending => owner-sorted
        k_of = sel[tsel, 1].astype(np.int64)   # slot (experts distinct)
        w_of = topk_w[tsel, :][np.arange(len(tsel)), k_of]
        tok_lists.append(tsel)
        wt_lists.append(w_of)

    c_max = max(len(tl) for tl in tok_lists)
    c_cap = max(C_CAP_DEFAULT, -(-c_max // 256) * 256)
    c_half = c_cap // 2

    # per-(expert, owner, half) positions + max chunk occupancy
    pair_max = 0
    pos_all, half_all = [], []
    for e in range(E):
        toks = tok_lists[e]
        own = owner[toks]
        cols = np.arange(len(toks))
        hh = (cols // c_half).astype(np.int64)
        pos = np.zeros(len(toks), np.int64)
        for o in range(NCORES):
            for h in range(2):
                mask = (own == o) & (hh == h)
                n = int(mask.sum())
                pos[mask] = np.arange(n)
                pair_max = max(pair_max, n)
        pos_all.append(pos)
        half_all.append(hh)
    p_cap = max(P_CAP_DEFAULT, -(-pair_max // 64) * 64)
    rows_h = NCORES * p_cap

    nc = _build(c_cap, p_cap)

    # host-side layouts (fp16 compute dtype)
    w1r = np.asarray(routed_w1, np.float16)              # [E, D, 2F]
    w2r = np.asarray(routed_w2, np.float16)              # [E, F, D]
    sw1r = np.asarray(shared_w1, np.float16)[0]          # [D, 2F]
    sw2r = np.asarray(shared_w2, np.float16)[0]          # [F, D]
    xr = x.astype(np.float16)                            # [T, D]

    def tile_w1(w):                            # [D,2F] -> [64, 8, 128, 128]
        return np.ascontiguousarray(
            w.reshape(D // 128, 128, 2 * F // 128, 128).transpose(2, 0, 1, 3))

    def col_bias(b):                           # [2F] -> [128, 64]
        return np.ascontiguousarray(
            np.asarray(b, np.float32).reshape(2 * F // 128, 128).T)

    sw1_t = tile_w1(sw1r)
    sb1_t = col_bias(np.asarray(shared_b1)[0])

    # absolute a2a_out row for each (token, slot)
    slot_rows = np.zeros((T, TOPK), np.int64)
    for e in range(E):
        toks = tok_lists[e]
        sel = (topk_idx[toks] == e)
        k_of = sel[:, 1].astype(np.int64)
        rows = half_all[e] * rows_h + e * p_cap + pos_all[e]
        slot_rows[toks, k_of] = rows

    in_maps = []
    for c in range(NCORES):
        e = c
        toks = tok_lists[e]
        wts = wt_lists[e]
        ce = len(toks)

        xg_a = np.zeros((D // 128, 128, c_cap), np.float16)
        xg_a[:, :, :ce] = xr[toks].T.reshape(D // 128, 128, ce)

        cw_a = np.zeros((c_cap, 1), np.float32)
        cw_a[:ce, 0] = wts

        scat_a = np.full((c_cap, 1), 2**31 - 1, np.int32)
        scat_a[:ce, 0] = (owner[toks] * p_cap + pos_all[e]).astype(np.int32)

        xs_a = np.ascontiguousarray(
            xr[c * TO:(c + 1) * TO].T.reshape(D // 128, 128, TO))

        g0 = slot_rows[c * TO:(c + 1) * TO, 0].astype(np.int32).reshape(TO, 1)
        g1 = slot_rows[c * TO:(c + 1) * TO, 1].astype(np.int32).reshape(TO, 1)

        in_maps.append({
            "xg": xg_a, "xs": xs_a,
            "w1": tile_w1(w1r[e]), "w2": np.ascontiguousarray(w2r[e]),
            "sw1": sw1_t, "sw2": sw2r,
            "b1": col_bias(np.asarray(routed_b1)[e]),
            "sb1": sb1_t,
            "b2": np.asarray(routed_b2, np.float32)[e].reshape(1, D).copy(),
            "sb2": np.asarray(shared_b2, np.float32)[0].reshape(1, D).copy(),
            "cw": cw_a, "scat": scat_a, "g0i": g0, "g1i": g1,
        })

    res = run_bass_kernel_spmd(nc, in_maps, list(range(NCORES)))
    return np.concatenate([res.results[c]["out"] for c in range(NCORES)],
                          axis=0)



# revision 2
# speedup vs baseline: 1.0557x; 1.0557x over previous
"""MoE (top-2 of 8 experts + 1 shared expert, SwiGLU FFN) on 8 TRN2 NeuronCores.

Strategy (expert-parallel, per the sharding hint):
  - Host computes the (tiny) gate: softmax top-2 over E=8 for T=8192 tokens,
    and the dispatch: per-expert gathered token lists plus scatter/gather
    index maps. >99.9% of FLOPs (the FFNs) run on device.
  - Core e receives the tokens routed to expert e (transposed, [D, c_alloc]),
    runs the SwiGLU FFN in ONE pass streaming exactly c_max columns, scales
    rows by the gate weight, and scatters rows into a single AllToAll
    dispatch buffer laid out by destination core (p_cap rows per (src,dst)
    block, exact — no rounding).
  - One AllToAll dispatches all routed outputs; it overlaps the shared-expert
    FFN which each core runs on its own T/8 token slice.
  - Combine on device: out[t] = shared(t) + r(t) where r(t) = contrib0+contrib1
    is accumulated during the indirect gather itself (compute_op=add).
  - Host concatenates the 8 [T/8, D] output slices.

Perf notes (vs the 2-half baseline at 1409 us):
  - The PE clock here is GPIO-power-capped at 13/16 x 2.4 = 1.95 GHz; the
    kernel is tensor-bound, so the main levers are cycle count and keeping
    the PE fed.
  - All DMAs are batched into large single transfers (w1: one 256KB load per
    f-chunk via host retiling, w2: one 2MB load per slab, index/weight
    vectors: one load each). The baseline's ~2000 x ~590ns serialized HWDGE
    issues kept the Sync engine 77% busy and starved the PE at phase starts.
  - During a collective, HWDGE model DMA starves almost completely; w1
    prefetch depth (bufs=6) gives the PE enough runway to ride out the
    single ~30us AllToAll window.
  - Stage-2 accumulators are fp16 (SBUF budget); bias adds are folded into
    the stage-2 PSUM drain.

Compute dtype fp16 (~6e-4 dot-product rel err vs fp32 reference, threshold
2e-2); PSUM accumulation fp32.
"""
import contextlib

import numpy as np

import concourse.bass as bass
import concourse.tile as tile
from concourse import bacc, mybir
from concourse.bass_utils import run_bass_kernel_spmd

# problem shape (hardcoded per contract)
T = 8192
D = 1024
F = 4096
E = 8
TOPK = 2
NCORES = 8
TO = T // NCORES          # tokens owned per core
KD = D // 128             # 8 contraction chunks for stage 1
MF = 2 * F // 128 // 2    # 32 a-chunks (b-chunks at +MF)
NSLAB = 4
PERS = MF // NSLAB        # 8 f-chunks per slab

F32 = mybir.dt.float32
F16 = mybir.dt.float16
I32 = mybir.dt.int32

_nc_cache: dict[tuple, object] = {}


def _chunk_slices(c_len):
    """Moving-dim chunks of <=512, each >=256 where possible."""
    out = []
    pos = 0
    rem = c_len
    while rem > 0:
        if rem > 512:
            w = 512 if rem - 512 >= 256 else 384
        else:
            w = rem
        out.append((pos, w))
        pos += w
        rem -= w
    return out


def _build(c_str, p_cap):
    key = (c_str, p_cap)
    if key in _nc_cache:
        return _nc_cache[key]

    NT = -(-c_str // 128)             # routed token tiles
    c_alloc = NT * 128
    NTS = TO // 128                   # shared token tiles (8)
    rows = NCORES * p_cap
    chunks_r = _chunk_slices(c_str)
    chunks_s = _chunk_slices(TO)

    nc = bacc.Bacc("TRN2", target_bir_lowering=False, debug=False,
                   num_devices=NCORES)

    def din(name, shape, dt):
        return nc.dram_tensor(name, shape, dt, kind="ExternalInput").ap()

    xg = din("xg", [KD, 128, c_alloc], F16)        # gathered tokens^T
    xs = din("xs", [KD, 128, TO], F16)             # owned tokens^T
    w1 = din("w1", [2 * MF, 128, KD, 128], F16)    # [mp, p, k, c]
    w2 = din("w2", [NSLAB, 128, PERS, D], F16)     # [q, p, fj, d]
    sw1 = din("sw1", [2 * MF, 128, KD, 128], F16)
    sw2 = din("sw2", [NSLAB, 128, PERS, D], F16)
    b1 = din("b1", [128, 2 * MF], F32)             # col m = chunk-m bias
    sb1 = din("sb1", [128, 2 * MF], F32)
    b2 = din("b2", [1, D], F32)
    sb2 = din("sb2", [1, D], F32)
    cwd = din("cw", [128, NT], F32)                # combine weights (col t)
    scat = din("scat", [128, NT], I32)             # scatter row in a2a_in
    g0i = din("g0i", [128, NTS], I32)              # gather rows in a2a_out
    g1i = din("g1i", [128, NTS], I32)
    out = nc.dram_tensor("out", [TO, D], F32, kind="ExternalOutput").ap()

    with tile.TileContext(nc) as tc:
        with contextlib.ExitStack() as ctx:
            sbuf = ctx.enter_context(tc.tile_pool(name="sbuf", bufs=1))
            psum = ctx.enter_context(tc.tile_pool(name="psum", bufs=2,
                                                  space="PSUM"))
            dpool = ctx.enter_context(tc.tile_pool(name="dram", bufs=1,
                                                   space="DRAM"))

            a2a_in = dpool.tile([rows, D], F16)
            a2a_out = dpool.tile([rows, D], F16)

            # resident small tensors (one batched DMA each)
            b1t = sbuf.tile([128, 2 * MF], F32, tag="b1t", name="b1t", bufs=1)
            sb1t = sbuf.tile([128, 2 * MF], F32, tag="sb1t", name="sb1t",
                             bufs=1)
            cwt = sbuf.tile([128, NT], F32, tag="cwt", name="cwt", bufs=1)
            sct = sbuf.tile([128, NT], I32, tag="sct", name="sct", bufs=1)
            g0t = sbuf.tile([128, NTS], I32, tag="g0t", name="g0t", bufs=1)
            g1t = sbuf.tile([128, NTS], I32, tag="g1t", name="g1t", bufs=1)
            nc.sync.dma_start(out=b1t[:], in_=b1[:])
            nc.sync.dma_start(out=sb1t[:], in_=sb1[:])
            nc.sync.dma_start(out=cwt[:], in_=cwd[:])
            nc.sync.dma_start(out=sct[:], in_=scat[:])
            nc.sync.dma_start(out=g0t[:], in_=g0i[:])
            nc.sync.dma_start(out=g1t[:], in_=g1i[:])
            b2t = sbuf.tile([128, D], F32, tag="b2t", name="b2t", bufs=1)
            sb2t = sbuf.tile([128, D], F32, tag="sb2t", name="sb2t", bufs=1)
            nc.gpsimd.dma_start(out=b2t[:], in_=b2.to_broadcast([128, D]))
            nc.gpsimd.dma_start(out=sb2t[:], in_=sb2.to_broadcast([128, D]))

            # gathered tokens (one batched DMA per contraction chunk)
            xk = []
            for k in range(KD):
                xt = sbuf.tile([128, c_alloc], F16, tag=f"xk{k}",
                               name=f"xk{k}", bufs=1)
                nc.sync.dma_start(out=xt[:], in_=xg[k])
                xk.append(xt)

            # stage-1 output tiles; zero the pad columns once so stage-2
            # matmuls on the last token tile read finite values
            g_tiles = []
            for fi in range(PERS):
                g_t = sbuf.tile([128, c_alloc], F16, tag=f"g{fi}",
                                name=f"g{fi}", bufs=1)
                if c_str < c_alloc:
                    nc.vector.memset(g_t[:, c_str:c_alloc], 0.0)
                g_tiles.append(g_t)

            y_tiles = [sbuf.tile([128, D], F16, tag=f"y{t}", name=f"y{t}",
                                 bufs=1) for t in range(NT)]

            def ffn(xk_tiles, w1d, w2d, b1t_, bias2_t, n_t, chunks):
                """One SwiGLU FFN pass; writes y_tiles[0..n_t-1] (fp16,
                bias2 folded in)."""
                for q in range(NSLAB):
                    w2t = sbuf.tile([128, PERS, D], F16, tag="w2",
                                    name="w2", bufs=2)
                    for fi in range(PERS):
                        mp = q * PERS + fi
                        w1a = sbuf.tile([128, KD, 128], F16, tag="w1a",
                                        name="w1a", bufs=6)
                        w1b = sbuf.tile([128, KD, 128], F16, tag="w1b",
                                        name="w1b", bufs=6)
                        nc.sync.dma_start(out=w1a[:], in_=w1d[mp])
                        nc.sync.dma_start(out=w1b[:], in_=w1d[mp + MF])
                        if fi == 0:
                            nc.sync.dma_start(out=w2t[:], in_=w2d[q])
                        g_t = g_tiles[fi]
                        for cs, cw in chunks:
                            ps_a = psum.tile([128, 512], F32, space="PSUM",
                                             tag="ps_a", name="ps_a", bufs=3)
                            ps_b = psum.tile([128, 512], F32, space="PSUM",
                                             tag="ps_b", name="ps_b", bufs=3)
                            for k in range(KD):
                                nc.tensor.matmul(out=ps_a[:, :cw],
                                                 lhsT=w1a[:, k, :],
                                                 rhs=xk_tiles[k][:, cs:cs + cw],
                                                 start=(k == 0),
                                                 stop=(k == KD - 1))
                            for k in range(KD):
                                nc.tensor.matmul(out=ps_b[:, :cw],
                                                 lhsT=w1b[:, k, :],
                                                 rhs=xk_tiles[k][:, cs:cs + cw],
                                                 start=(k == 0),
                                                 stop=(k == KD - 1))
                            t_a = sbuf.tile([128, 512], F16, tag="t_a",
                                            name="t_a", bufs=2)
                            t_b = sbuf.tile([128, 512], F16, tag="t_b",
                                            name="t_b", bufs=2)
                            nc.scalar.activation(
                                t_a[:, :cw], ps_a[:, :cw],
                                mybir.ActivationFunctionType.Silu,
                                bias=b1t_[:, mp:mp + 1])
                            nc.scalar.activation(
                                t_b[:, :cw], ps_b[:, :cw],
                                mybir.ActivationFunctionType.Identity,
                                bias=b1t_[:, mp + MF:mp + MF + 1])
                            nc.vector.tensor_mul(g_t[:, cs:cs + cw],
                                                 t_a[:, :cw], t_b[:, :cw])
                    # stage-2 partial: y (+)= g_slab.T @ w2_slab
                    for t in range(n_t):
                        ts = slice(t * 128, (t + 1) * 128)
                        for dd in range(D // 512):
                            ds = slice(dd * 512, (dd + 1) * 512)
                            ps_y = psum.tile([128, 512], F32, space="PSUM",
                                             tag="ps_y", name="ps_y", bufs=2)
                            for fi in range(PERS):
                                nc.tensor.matmul(out=ps_y[:],
                                                 lhsT=g_tiles[fi][:, ts],
                                                 rhs=w2t[:, fi, ds],
                                                 start=(fi == 0),
                                                 stop=(fi == PERS - 1))
                            yt = y_tiles[t]
                            if q == 0:
                                nc.vector.tensor_add(yt[:, ds], ps_y[:],
                                                     bias2_t[:, ds])
                            else:
                                nc.vector.tensor_add(yt[:, ds], yt[:, ds],
                                                     ps_y[:])

            # ---------------- routed expert (single pass) ------------------
            ffn(xk, w1, w2, b1t, b2t, NT, chunks_r)

            # finalize: scale by combine weight, scatter into a2a_in
            for t in range(NT):
                yh = sbuf.tile([128, D], F16, tag="yh", name="yh", bufs=1)
                nc.vector.tensor_scalar_mul(yh[:], y_tiles[t][:],
                                            cwt[:, t:t + 1])
                nc.gpsimd.indirect_dma_start(
                    out=a2a_in[:],
                    out_offset=bass.IndirectOffsetOnAxis(ap=sct[:, t:t + 1],
                                                         axis=0),
                    in_=yh[:],
                    in_offset=None,
                    bounds_check=rows - 1,
                    oob_is_err=False,
                )
            nc.gpsimd.collective_compute(
                "AllToAll",
                mybir.AluOpType.bypass,
                replica_groups=[list(range(NCORES))],
                ins=[a2a_in[:].opt()],
                outs=[a2a_out[:].opt()],
            )

            # combine gathers: r[t] = contrib0 + contrib1 (accumulated in DMA)
            r_tiles = []
            for t in range(NTS):
                rt = sbuf.tile([128, D], F16, tag=f"r{t}", name=f"r{t}",
                               bufs=1)
                nc.gpsimd.indirect_dma_start(
                    out=rt[:], out_offset=None, in_=a2a_out[:],
                    in_offset=bass.IndirectOffsetOnAxis(ap=g0t[:, t:t + 1],
                                                        axis=0))
                nc.gpsimd.indirect_dma_start(
                    out=rt[:], out_offset=None, in_=a2a_out[:],
                    in_offset=bass.IndirectOffsetOnAxis(ap=g1t[:, t:t + 1],
                                                        axis=0),
                    compute_op=mybir.AluOpType.add)
                r_tiles.append(rt)

            # ---------------- shared expert on owned tokens (overlaps) -----
            xsk = []
            for k in range(KD):
                xt = sbuf.tile([128, c_alloc], F16, tag=f"xk{k}",
                               name=f"xk{k}", bufs=1)
                nc.sync.dma_start(out=xt[:, :TO], in_=xs[k])
                xsk.append(xt)
            ffn(xsk, sw1, sw2, sb1t, sb2t, NTS, chunks_s)

            # final combine + output
            for t in range(NTS):
                yo = sbuf.tile([128, D], F32, tag="yo", name="yo", bufs=1)
                nc.vector.tensor_add(yo[:], y_tiles[t][:], r_tiles[t][:])
                nc.sync.dma_start(out=out[t * 128:(t + 1) * 128, :],
                                  in_=yo[:])

    nc.compile()
    _nc_cache[key] = nc
    return nc


def _route(x, gate_w, gate_b):
    """Host gate: softmax top-2 (float64 for stable ordering)."""
    logits = (x.astype(np.float64) @ gate_w.astype(np.float64)
              + gate_b.astype(np.float64))
    m = logits.max(axis=-1, keepdims=True)
    p = np.exp(logits - m)
    p /= p.sum(axis=-1, keepdims=True)
    order = np.argsort(-p, axis=-1)
    idx = order[:, :TOPK]                      # [T, 2]
    wts = np.take_along_axis(p, idx, axis=-1)  # [T, 2]
    return idx, wts.astype(np.float32)


def kernel(x, gate_w, gate_b, shared_w1, shared_b1, shared_w2, shared_b2,
           routed_w1, routed_b1, routed_w2, routed_b2):
    x = np.asarray(x, dtype=np.float32)
    topk_idx, topk_w = _route(x, np.asarray(gate_w), np.asarray(gate_b))

    owner = np.arange(T) // TO                 # owning core per token

    # per-expert dispatch lists, ordered by (owner, token)
    tok_lists, wt_lists, pos_lists = [], [], []
    p_cap = 0
    for e in range(E):
        sel = (topk_idx == e)                  # [T, 2]
        tsel = np.nonzero(sel.any(axis=1))[0]  # ascending => owner-sorted
        k_of = sel[tsel, 1].astype(np.int64)   # slot (experts distinct)
        w_of = topk_w[tsel, :][np.arange(len(tsel)), k_of]
        own = owner[tsel]
        pos = np.zeros(len(tsel), np.int64)
        for o in range(NCORES):
            mask = own == o
            n = int(mask.sum())
            pos[mask] = np.arange(n)
            p_cap = max(p_cap, n)
        tok_lists.append(tsel)
        wt_lists.append(w_of)
        pos_lists.append(pos)

    c_max = max(len(tl) for tl in tok_lists)
    NT = -(-c_max // 128)
    c_alloc = NT * 128
    rows = NCORES * p_cap

    nc = _build(c_max, p_cap)

    # host-side layouts (fp16 compute dtype)
    w1r = np.asarray(routed_w1, np.float16)              # [E, D, 2F]
    w2r = np.asarray(routed_w2, np.float16)              # [E, F, D]
    sw1r = np.asarray(shared_w1, np.float16)[0]          # [D, 2F]
    sw2r = np.asarray(shared_w2, np.float16)[0]          # [F, D]
    xr = x.astype(np.float16)                            # [T, D]

    def tile_w1(w):                # [D,2F] -> [mp=64, p=128, k=8, c=128]
        return np.ascontiguousarray(
            w.reshape(KD, 128, 2 * MF, 128).transpose(2, 1, 0, 3))

    def tile_w2(w):                # [F,D] -> [q=4, p=128, fj=8, d=1024]
        return np.ascontiguousarray(
            w.reshape(NSLAB, PERS, 128, D).transpose(0, 2, 1, 3))

    def col_bias(b):               # [2F] -> [128, 64]
        return np.ascontiguousarray(
            np.asarray(b, np.float32).reshape(2 * MF, 128).T)

    sw1_t = tile_w1(sw1r)
    sw2_t = tile_w2(sw2r)
    sb1_t = col_bias(np.asarray(shared_b1)[0])

    # a2a_out row for each (token, slot): src_expert * p_cap + pos
    slot_rows = np.zeros((T, TOPK), np.int64)
    for e in range(E):
        toks = tok_lists[e]
        sel = (topk_idx[toks] == e)
        k_of = sel[:, 1].astype(np.int64)
        slot_rows[toks, k_of] = e * p_cap + pos_lists[e]

    in_maps = []
    for c in range(NCORES):
        e = c
        toks = tok_lists[e]
        wts = wt_lists[e]
        ce = len(toks)

        xg_a = np.zeros((KD, 128, c_alloc), np.float16)
        xg_a[:, :, :ce] = xr[toks].T.reshape(KD, 128, ce)

        cw_a = np.zeros((NT * 128,), np.float32)
        cw_a[:ce] = wts
        cw_t = np.ascontiguousarray(cw_a.reshape(NT, 128).T)

        scat_a = np.full((NT * 128,), 2**31 - 1, np.int32)
        scat_a[:ce] = (owner[toks] * p_cap + pos_lists[e]).astype(np.int32)
        scat_t = np.ascontiguousarray(scat_a.reshape(NT, 128).T)

        xs_a = np.ascontiguousarray(
            xr[c * TO:(c + 1) * TO].T.reshape(KD, 128, TO))

        g0 = slot_rows[c * TO:(c + 1) * TO, 0].astype(np.int32)
        g1 = slot_rows[c * TO:(c + 1) * TO, 1].astype(np.int32)
        g0_t = np.ascontiguousarray(g0.reshape(TO // 128, 128).T)
        g1_t = np.ascontiguousarray(g1.reshape(TO // 128, 128).T)

        in_maps.append({
            "xg": xg_a, "xs": xs_a,
            "w1": tile_w1(w1r[e]), "w2": tile_w2(w2r[e]),
            "sw1": sw1_t, "sw2": sw2_t,
            "b1": col_bias(np.asarray(routed_b1)[e]),
            "sb1": sb1_t,
            "b2": np.asarray(routed_b2, np.float32)[e].reshape(1, D).copy(),
            "sb2": np.asarray(shared_b2, np.float32)[0].reshape(1, D).copy(),
            "cw": cw_t, "scat": scat_t, "g0i": g0_t, "g1i": g1_t,
        })

    res = run_bass_kernel_spmd(nc, in_maps, list(range(NCORES)))
    return np.concatenate([res.results[c]["out"] for c in range(NCORES)],
                          axis=0)


# revision 8
# speedup vs baseline: 1.0721x; 1.0155x over previous
"""MoE (top-2 of 8 experts + 1 shared expert, SwiGLU FFN) on 8 TRN2 NeuronCores.

Strategy (expert-parallel, per the sharding hint):
  - Host computes the (tiny) gate: softmax top-2 over E=8 for T=8192 tokens,
    and the dispatch: per-expert gathered token lists plus scatter/gather
    index maps. >99.9% of FLOPs (the FFNs) run on device.
  - Core e receives the tokens routed to expert e (transposed, [D, c_alloc]),
    runs the SwiGLU FFN in ONE pass streaming exactly c_max columns, scales
    rows by the gate weight, and scatters rows into a single AllToAll
    dispatch buffer laid out by destination core (p_cap rows per (src,dst)
    block, exact — no rounding).
  - One AllToAll dispatches all routed outputs; it overlaps the shared-expert
    FFN which each core runs on its own T/8 token slice.
  - Combine on device: out[t] = shared(t) + r(t) where r(t) = contrib0+contrib1
    is accumulated during the indirect gather itself (compute_op=add).
  - Host concatenates the 8 [T/8, D] output slices.

Perf notes (vs the 2-half baseline at 1409 us):
  - The PE clock here is GPIO-power-capped at 13/16 x 2.4 = 1.95 GHz; the
    kernel is tensor-bound, so the main levers are cycle count and keeping
    the PE fed.
  - All DMAs are batched into large single transfers (w1: one 256KB load per
    f-chunk via host retiling, w2: one 2MB load per slab, index/weight
    vectors: one load each). The baseline's ~2000 x ~590ns serialized HWDGE
    issues kept the Sync engine 77% busy and starved the PE at phase starts.
  - During a collective, HWDGE model DMA starves almost completely; w1
    prefetch depth (bufs=6) gives the PE enough runway to ride out the
    single ~30us AllToAll window.
  - Stage-2 accumulators are fp16 (SBUF budget); bias adds are folded into
    the stage-2 PSUM drain.

Compute dtype fp16 (~6e-4 dot-product rel err vs fp32 reference, threshold
2e-2); PSUM accumulation fp32.
"""
import contextlib

import numpy as np

import concourse.bass as bass
import concourse.tile as tile
from concourse import bacc, mybir
from concourse.bass_utils import run_bass_kernel_spmd

# problem shape (hardcoded per contract)
T = 8192
D = 1024
F = 4096
E = 8
TOPK = 2
NCORES = 8
TO = T // NCORES          # tokens owned per core
KD = D // 128             # 8 contraction chunks for stage 1
MF = 2 * F // 128 // 2    # 32 a-chunks (b-chunks at +MF)
NSLAB = 4
PERS = MF // NSLAB        # 8 f-chunks per slab

F32 = mybir.dt.float32
F16 = mybir.dt.float16
I32 = mybir.dt.int32

_nc_cache: dict[tuple, object] = {}


def _chunk_slices(c_len):
    """Moving-dim chunks of <=512, each >=256 where possible."""
    out = []
    pos = 0
    rem = c_len
    while rem > 0:
        if rem > 512:
            w = 512 if rem - 512 >= 256 else 384
        else:
            w = rem
        out.append((pos, w))
        pos += w
        rem -= w
    return out


def _build(c_str, p_cap):
    key = (c_str, p_cap)
    if key in _nc_cache:
        return _nc_cache[key]

    NT = -(-c_str // 128)             # routed token tiles
    c_alloc = NT * 128
    NTS = TO // 128                   # shared token tiles (8)
    rows = NCORES * p_cap
    chunks_r = _chunk_slices(c_str)
    chunks_s = _chunk_slices(TO)

    nc = bacc.Bacc("TRN2", target_bir_lowering=False, debug=False,
                   num_devices=NCORES)

    def din(name, shape, dt):
        return nc.dram_tensor(name, shape, dt, kind="ExternalInput").ap()

    xg = din("xg", [KD, 128, c_alloc], F16)        # gathered tokens^T
    xs = din("xs", [KD, 128, TO], F16)             # owned tokens^T
    w1 = din("w1", [2 * MF, 128, KD, 128], F16)    # [mp, p, k, c]
    w2 = din("w2", [NSLAB, 128, PERS, D], F16)     # [q, p, fj, d]
    sw1 = din("sw1", [2 * MF, 128, KD, 128], F16)
    sw2 = din("sw2", [NSLAB, 128, PERS, D], F16)
    b1 = din("b1", [128, 2 * MF], F32)             # col m = chunk-m bias
    sb1 = din("sb1", [128, 2 * MF], F32)
    b2 = din("b2", [1, D], F32)
    sb2 = din("sb2", [1, D], F32)
    cwd = din("cw", [128, NT], F32)                # combine weights (col t)
    scat = din("scat", [128, NT], I32)             # scatter row in a2a_in
    g0i = din("g0i", [128, NTS], I32)              # gather rows in a2a_out
    g1i = din("g1i", [128, NTS], I32)
    out = nc.dram_tensor("out", [TO, D], F32, kind="ExternalOutput").ap()

    with tile.TileContext(nc) as tc:
        with contextlib.ExitStack() as ctx:
            sbuf = ctx.enter_context(tc.tile_pool(name="sbuf", bufs=1))
            psum = ctx.enter_context(tc.tile_pool(name="psum", bufs=2,
                                                  space="PSUM"))
            dpool = ctx.enter_context(tc.tile_pool(name="dram", bufs=1,
                                                   space="DRAM"))

            a2a_in = dpool.tile([rows, D], F16)
            a2a_out = dpool.tile([rows, D], F16)

            # resident small tensors (one batched DMA each)
            b1t = sbuf.tile([128, 2 * MF], F32, tag="b1t", name="b1t", bufs=1)
            sb1t = sbuf.tile([128, 2 * MF], F32, tag="sb1t", name="sb1t",
                             bufs=1)
            cwt = sbuf.tile([128, NT], F32, tag="cwt", name="cwt", bufs=1)
            sct = sbuf.tile([128, NT], I32, tag="sct", name="sct", bufs=1)
            g0t = sbuf.tile([128, NTS], I32, tag="g0t", name="g0t", bufs=1)
            g1t = sbuf.tile([128, NTS], I32, tag="g1t", name="g1t", bufs=1)
            nc.sync.dma_start(out=b1t[:], in_=b1[:])
            nc.sync.dma_start(out=sb1t[:], in_=sb1[:])
            b2t = sbuf.tile([128, D], F32, tag="b2t", name="b2t", bufs=1)
            sb2t = sbuf.tile([128, D], F32, tag="sb2t", name="sb2t", bufs=1)
            nc.gpsimd.dma_start(out=b2t[:], in_=b2.to_broadcast([128, D]))
            nc.gpsimd.dma_start(out=sb2t[:], in_=sb2.to_broadcast([128, D]))

            # gathered tokens: the first stage-1 chunk's columns land first
            # so the PE can start ~25us earlier; the remainder follows
            c0 = chunks_r[0][1]
            xk = []
            for k in range(KD):
                xt = sbuf.tile([128, c_alloc], F16, tag=f"xk{k}",
                               name=f"xk{k}", bufs=1)
                nc.sync.dma_start(out=xt[:, :c0], in_=xg[k][:, :c0])
                xk.append(xt)
            # first f-chunk's weights ahead of the xk remainder in the DMA
            # queue: the PE's first matmul needs only these + the c0 slices
            w1a0 = sbuf.tile([128, KD, 128], F16, tag="w1a", name="w1a",
                             bufs=6)
            w1b0 = sbuf.tile([128, KD, 128], F16, tag="w1b", name="w1b",
                             bufs=6)
            nc.sync.dma_start(out=w1a0[:], in_=w1[0])
            nc.sync.dma_start(out=w1b0[:], in_=w1[MF])
            for k in range(KD):
                nc.sync.dma_start(out=xk[k][:, c0:], in_=xg[k][:, c0:])

            # stage-1 output tiles; zero the pad columns once so stage-2
            # matmuls on the last token tile read finite values
            g_tiles = []
            for fi in range(PERS):
                g_t = sbuf.tile([128, c_alloc], F16, tag=f"g{fi}",
                                name=f"g{fi}", bufs=1)
                if c_str < c_alloc:
                    nc.vector.memset(g_t[:, c_str:c_alloc], 0.0)
                g_tiles.append(g_t)

            y_tiles = [sbuf.tile([128, D], F16, tag=f"y{t}", name=f"y{t}",
                                 bufs=1) for t in range(NT)]

            def ffn(xk_tiles, w1d, w2d, b1t_, bias2_t, n_t, chunks,
                    w1_pre=None, final=None):
                """One SwiGLU FFN pass; writes y_tiles[0..n_t-1] (fp16,
                bias2 folded in). final=(r_tiles, out_ap) additionally emits
                the per-tile combine + output DMA inline with the last
                slab's stage-2."""
                for q in range(NSLAB):
                    w2t = sbuf.tile([128, PERS, D], F16, tag="w2",
                                    name="w2", bufs=2)
                    for fi in range(PERS):
                        mp = q * PERS + fi
                        if fi == 0 and q == 0 and w1_pre is not None:
                            w1a, w1b = w1_pre
                        else:
                            w1a = sbuf.tile([128, KD, 128], F16, tag="w1a",
                                            name="w1a", bufs=6)
                            w1b = sbuf.tile([128, KD, 128], F16, tag="w1b",
                                            name="w1b", bufs=6)
                            nc.sync.dma_start(out=w1a[:], in_=w1d[mp])
                            nc.sync.dma_start(out=w1b[:], in_=w1d[mp + MF])
                        if fi == 1:
                            nc.sync.dma_start(out=w2t[:], in_=w2d[q])
                        g_t = g_tiles[fi]
                        for cs, cw in chunks:
                            ps_a = psum.tile([128, 512], F32, space="PSUM",
                                             tag="ps_a", name="ps_a", bufs=3)
                            ps_b = psum.tile([128, 512], F32, space="PSUM",
                                             tag="ps_b", name="ps_b", bufs=3)
                            for k in range(KD):
                                nc.tensor.matmul(out=ps_a[:, :cw],
                                                 lhsT=w1a[:, k, :],
                                                 rhs=xk_tiles[k][:, cs:cs + cw],
                                                 start=(k == 0),
                                                 stop=(k == KD - 1))
                            for k in range(KD):
                                nc.tensor.matmul(out=ps_b[:, :cw],
                                                 lhsT=w1b[:, k, :],
                                                 rhs=xk_tiles[k][:, cs:cs + cw],
                                                 start=(k == 0),
                                                 stop=(k == KD - 1))
                            t_a = sbuf.tile([128, 512], F16, tag="t_a",
                                            name="t_a", bufs=2)
                            t_b = sbuf.tile([128, 512], F16, tag="t_b",
                                            name="t_b", bufs=2)
                            nc.scalar.activation(
                                t_a[:, :cw], ps_a[:, :cw],
                                mybir.ActivationFunctionType.Silu,
                                bias=b1t_[:, mp:mp + 1])
                            nc.scalar.activation(
                                t_b[:, :cw], ps_b[:, :cw],
                                mybir.ActivationFunctionType.Identity,
                                bias=b1t_[:, mp + MF:mp + MF + 1])
                            nc.vector.tensor_mul(g_t[:, cs:cs + cw],
                                                 t_a[:, :cw], t_b[:, :cw])
                    # stage-2 partial: y (+)= g_slab.T @ w2_slab
                    for t in range(n_t):
                        ts = slice(t * 128, (t + 1) * 128)
                        for dd in range(D // 512):
                            ds = slice(dd * 512, (dd + 1) * 512)
                            ps_y = psum.tile([128, 512], F32, space="PSUM",
                                             tag="ps_y", name="ps_y", bufs=2)
                            for fi in range(PERS):
                                nc.tensor.matmul(out=ps_y[:],
                                                 lhsT=g_tiles[fi][:, ts],
                                                 rhs=w2t[:, fi, ds],
                                                 start=(fi == 0),
                                                 stop=(fi == PERS - 1))
                            yt = y_tiles[t]
                            if q == 0:
                                nc.vector.tensor_add(yt[:, ds], ps_y[:],
                                                     bias2_t[:, ds])
                            else:
                                nc.vector.tensor_add(yt[:, ds], yt[:, ds],
                                                     ps_y[:])
                            if q == NSLAB - 1 and final is not None:
                                r_t, out_ap = final
                                if dd == 0:
                                    yo = sbuf.tile([128, D], F32, tag="yo",
                                                   name="yo", bufs=2)
                                nc.vector.tensor_add(yo[:, ds], yt[:, ds],
                                                     r_t[t][:, ds])
                        if q == NSLAB - 1 and final is not None:
                            nc.sync.dma_start(
                                out=final[1][t * 128:(t + 1) * 128, :],
                                in_=yo[:])

            # ---------------- routed expert (single pass) ------------------
            ffn(xk, w1, w2, b1t, b2t, NT, chunks_r, w1_pre=(w1a0, w1b0))
            nc.sync.dma_start(out=cwt[:], in_=cwd[:])
            nc.sync.dma_start(out=sct[:], in_=scat[:])
            nc.sync.dma_start(out=g0t[:], in_=g0i[:])
            nc.sync.dma_start(out=g1t[:], in_=g1i[:])

            # finalize: scale by combine weight, scatter into a2a_in
            for t in range(NT):
                yh = sbuf.tile([128, D], F16, tag="yh", name="yh", bufs=1)
                nc.vector.tensor_scalar_mul(yh[:], y_tiles[t][:],
                                            cwt[:, t:t + 1])
                nc.gpsimd.indirect_dma_start(
                    out=a2a_in[:],
                    out_offset=bass.IndirectOffsetOnAxis(ap=sct[:, t:t + 1],
                                                         axis=0),
                    in_=yh[:],
                    in_offset=None,
                    bounds_check=rows - 1,
                    oob_is_err=False,
                )
            nc.gpsimd.collective_compute(
                "AllToAll",
                mybir.AluOpType.bypass,
                replica_groups=[list(range(NCORES))],
                ins=[a2a_in[:].opt()],
                outs=[a2a_out[:].opt()],
            )

            # combine gathers: r[t] = contrib0 + contrib1 (accumulated in DMA)
            r_tiles = []
            for t in range(NTS):
                rt = sbuf.tile([128, D], F16, tag=f"r{t}", name=f"r{t}",
                               bufs=1)
                nc.gpsimd.indirect_dma_start(
                    out=rt[:], out_offset=None, in_=a2a_out[:],
                    in_offset=bass.IndirectOffsetOnAxis(ap=g0t[:, t:t + 1],
                                                        axis=0))
                nc.gpsimd.indirect_dma_start(
                    out=rt[:], out_offset=None, in_=a2a_out[:],
                    in_offset=bass.IndirectOffsetOnAxis(ap=g1t[:, t:t + 1],
                                                        axis=0),
                    compute_op=mybir.AluOpType.add)
                r_tiles.append(rt)

            # ---------------- shared expert on owned tokens (overlaps) -----
            xsk = []
            for k in range(KD):
                xt = sbuf.tile([128, c_alloc], F16, tag=f"xk{k}",
                               name=f"xk{k}", bufs=1)
                nc.sync.dma_start(out=xt[:, :TO], in_=xs[k])
                xsk.append(xt)
            ffn(xsk, sw1, sw2, sb1t, sb2t, NTS, chunks_s,
                final=(r_tiles, out))

    nc.compile()
    _nc_cache[key] = nc
    return nc


def _route(x, gate_w, gate_b):
    """Host gate: softmax top-2 (float64 for stable ordering)."""
    logits = (x.astype(np.float64) @ gate_w.astype(np.float64)
              + gate_b.astype(np.float64))
    m = logits.max(axis=-1, keepdims=True)
    p = np.exp(logits - m)
    p /= p.sum(axis=-1, keepdims=True)
    order = np.argsort(-p, axis=-1)
    idx = order[:, :TOPK]                      # [T, 2]
    wts = np.take_along_axis(p, idx, axis=-1)  # [T, 2]
    return idx, wts.astype(np.float32)


def kernel(x, gate_w, gate_b, shared_w1, shared_b1, shared_w2, shared_b2,
           routed_w1, routed_b1, routed_w2, routed_b2):
    x = np.asarray(x, dtype=np.float32)
    topk_idx, topk_w = _route(x, np.asarray(gate_w), np.asarray(gate_b))

    owner = np.arange(T) // TO                 # owning core per token

    # per-expert dispatch lists, ordered by (owner, token)
    tok_lists, wt_lists, pos_lists = [], [], []
    p_cap = 0
    for e in range(E):
        sel = (topk_idx == e)                  # [T, 2]
        tsel = np.nonzero(sel.any(axis=1))[0]  # ascending => owner-sorted
        k_of = sel[tsel, 1].astype(np.int64)   # slot (experts distinct)
        w_of = topk_w[tsel, :][np.arange(len(tsel)), k_of]
        own = owner[tsel]
        pos = np.zeros(len(tsel), np.int64)
        for o in range(NCORES):
            mask = own == o
            n = int(mask.sum())
            pos[mask] = np.arange(n)
            p_cap = max(p_cap, n)
        tok_lists.append(tsel)
        wt_lists.append(w_of)
        pos_lists.append(pos)

    c_max = max(len(tl) for tl in tok_lists)
    NT = -(-c_max // 128)
    c_alloc = NT * 128
    rows = NCORES * p_cap

    nc = _build(c_max, p_cap)

    # host-side layouts (fp16 compute dtype)
    w1r = np.asarray(routed_w1, np.float16)              # [E, D, 2F]
    w2r = np.asarray(routed_w2, np.float16)              # [E, F, D]
    sw1r = np.asarray(shared_w1, np.float16)[0]          # [D, 2F]
    sw2r = np.asarray(shared_w2, np.float16)[0]          # [F, D]
    xr = x.astype(np.float16)                            # [T, D]

    def tile_w1(w):                # [D,2F] -> [mp=64, p=128, k=8, c=128]
        return np.ascontiguousarray(
            w.reshape(KD, 128, 2 * MF, 128).transpose(2, 1, 0, 3))

    def tile_w2(w):                # [F,D] -> [q=4, p=128, fj=8, d=1024]
        return np.ascontiguousarray(
            w.reshape(NSLAB, PERS, 128, D).transpose(0, 2, 1, 3))

    def col_bias(b):               # [2F] -> [128, 64]
        return np.ascontiguousarray(
            np.asarray(b, np.float32).reshape(2 * MF, 128).T)

    sw1_t = tile_w1(sw1r)
    sw2_t = tile_w2(sw2r)
    sb1_t = col_bias(np.asarray(shared_b1)[0])

    # a2a_out row for each (token, slot): src_expert * p_cap + pos
    slot_rows = np.zeros((T, TOPK), np.int64)
    for e in range(E):
        toks = tok_lists[e]
        sel = (topk_idx[toks] == e)
        k_of = sel[:, 1].astype(np.int64)
        slot_rows[toks, k_of] = e * p_cap + pos_lists[e]

    in_maps = []
    for c in range(NCORES):
        e = c
        toks = tok_lists[e]
        wts = wt_lists[e]
        ce = len(toks)

        xg_a = np.zeros((KD, 128, c_alloc), np.float16)
        xg_a[:, :, :ce] = xr[toks].T.reshape(KD, 128, ce)

        cw_a = np.zeros((NT * 128,), np.float32)
        cw_a[:ce] = wts
        cw_t = np.ascontiguousarray(cw_a.reshape(NT, 128).T)

        scat_a = np.full((NT * 128,), 2**31 - 1, np.int32)
        scat_a[:ce] = (owner[toks] * p_cap + pos_lists[e]).astype(np.int32)
        scat_t = np.ascontiguousarray(scat_a.reshape(NT, 128).T)

        xs_a = np.ascontiguousarray(
            xr[c * TO:(c + 1) * TO].T.reshape(KD, 128, TO))

        g0 = slot_rows[c * TO:(c + 1) * TO, 0].astype(np.int32)
        g1 = slot_rows[c * TO:(c + 1) * TO, 1].astype(np.int32)
        g0_t = np.ascontiguousarray(g0.reshape(TO // 128, 128).T)
        g1_t = np.ascontiguousarray(g1.reshape(TO // 128, 128).T)

        in_maps.append({
            "xg": xg_a, "xs": xs_a,
            "w1": tile_w1(w1r[e]), "w2": tile_w2(w2r[e]),
            "sw1": sw1_t, "sw2": sw2_t,
            "b1": col_bias(np.asarray(routed_b1)[e]),
            "sb1": sb1_t,
            "b2": np.asarray(routed_b2, np.float32)[e].reshape(1, D).copy(),
            "sb2": np.asarray(shared_b2, np.float32)[0].reshape(1, D).copy(),
            "cw": cw_t, "scat": scat_t, "g0i": g0_t, "g1i": g1_t,
        })

    res = run_bass_kernel_spmd(nc, in_maps, list(range(NCORES)))
    return np.concatenate([res.results[c]["out"] for c in range(NCORES)],
                          axis=0)


# revision 9
# speedup vs baseline: 1.0756x; 1.0033x over previous
"""MoE (top-2 of 8 experts + 1 shared expert, SwiGLU FFN) on 8 TRN2 NeuronCores.

Strategy (expert-parallel, per the sharding hint):
  - Host computes the (tiny) gate: softmax top-2 over E=8 for T=8192 tokens,
    and the dispatch: per-expert gathered token lists plus scatter/gather
    index maps. >99.9% of FLOPs (the FFNs) run on device.
  - Core e receives the tokens routed to expert e (transposed, [D, c_alloc]),
    runs the SwiGLU FFN in ONE pass streaming exactly c_max columns, scales
    rows by the gate weight, and scatters rows into a single AllToAll
    dispatch buffer laid out by destination core (p_cap rows per (src,dst)
    block, exact — no rounding).
  - One AllToAll dispatches all routed outputs; it overlaps the shared-expert
    FFN which each core runs on its own T/8 token slice.
  - Combine on device: out[t] = shared(t) + r(t) where r(t) = contrib0+contrib1
    is accumulated during the indirect gather itself (compute_op=add).
  - Host concatenates the 8 [T/8, D] output slices.

Perf notes (vs the 2-half baseline at 1409 us):
  - The PE clock here is GPIO-power-capped at 13/16 x 2.4 = 1.95 GHz; the
    kernel is tensor-bound, so the main levers are cycle count and keeping
    the PE fed.
  - All DMAs are batched into large single transfers (w1: one 256KB load per
    f-chunk via host retiling, w2: one 2MB load per slab, index/weight
    vectors: one load each). The baseline's ~2000 x ~590ns serialized HWDGE
    issues kept the Sync engine 77% busy and starved the PE at phase starts.
  - During a collective, HWDGE model DMA starves almost completely; w1
    prefetch depth (bufs=6) gives the PE enough runway to ride out the
    single ~30us AllToAll window.
  - Stage-2 accumulators are fp16 (SBUF budget); bias adds are folded into
    the stage-2 PSUM drain.

Compute dtype fp16 (~6e-4 dot-product rel err vs fp32 reference, threshold
2e-2); PSUM accumulation fp32.
"""
import contextlib

import numpy as np

import concourse.bass as bass
import concourse.tile as tile
from concourse import bacc, mybir
from concourse.bass_utils import run_bass_kernel_spmd

# problem shape (hardcoded per contract)
T = 8192
D = 1024
F = 4096
E = 8
TOPK = 2
NCORES = 8
TO = T // NCORES          # tokens owned per core
KD = D // 128             # 8 contraction chunks for stage 1
MF = 2 * F // 128 // 2    # 32 a-chunks (b-chunks at +MF)
NSLAB = 4
PERS = MF // NSLAB        # 8 f-chunks per slab

F32 = mybir.dt.float32
F16 = mybir.dt.float16
I32 = mybir.dt.int32

_nc_cache: dict[tuple, object] = {}


def _chunk_slices(c_len):
    """Moving-dim chunks of <=512, each >=256 where possible."""
    out = []
    pos = 0
    rem = c_len
    while rem > 0:
        if rem > 512:
            w = 512 if rem - 512 >= 256 else 384
        else:
            w = rem
        out.append((pos, w))
        pos += w
        rem -= w
    return out


def _build(c_str, p_cap):
    key = (c_str, p_cap)
    if key in _nc_cache:
        return _nc_cache[key]

    NT = -(-c_str // 128)             # routed token tiles
    c_alloc = NT * 128
    NTS = TO // 128                   # shared token tiles (8)
    rows = NCORES * p_cap
    chunks_r = _chunk_slices(c_str)
    chunks_s = _chunk_slices(TO)

    nc = bacc.Bacc("TRN2", target_bir_lowering=False, debug=False,
                   num_devices=NCORES)

    def din(name, shape, dt):
        return nc.dram_tensor(name, shape, dt, kind="ExternalInput").ap()

    xg = din("xg", [KD, 128, c_alloc], F16)        # gathered tokens^T
    xs = din("xs", [KD, 128, TO], F16)             # owned tokens^T
    w1 = din("w1", [2 * MF, 128, KD, 128], F16)    # [mp, p, k, c]
    w2 = din("w2", [NSLAB, 128, PERS, D], F16)     # [q, p, fj, d]
    sw1 = din("sw1", [2 * MF, 128, KD, 128], F16)
    sw2 = din("sw2", [NSLAB, 128, PERS, D], F16)
    b1 = din("b1", [128, 2 * MF], F32)             # col m = chunk-m bias
    sb1 = din("sb1", [128, 2 * MF], F32)
    b2 = din("b2", [1, D], F32)
    sb2 = din("sb2", [1, D], F32)
    cwd = din("cw", [128, NT], F32)                # combine weights (col t)
    scat = din("scat", [128, NT], I32)             # scatter row in a2a_in
    g0i = din("g0i", [128, NTS], I32)              # gather rows in a2a_out
    g1i = din("g1i", [128, NTS], I32)
    out = nc.dram_tensor("out", [TO, D], F32, kind="ExternalOutput").ap()

    with tile.TileContext(nc) as tc:
        with contextlib.ExitStack() as ctx:
            sbuf = ctx.enter_context(tc.tile_pool(name="sbuf", bufs=1))
            psum = ctx.enter_context(tc.tile_pool(name="psum", bufs=2,
                                                  space="PSUM"))
            dpool = ctx.enter_context(tc.tile_pool(name="dram", bufs=1,
                                                   space="DRAM"))

            a2a_in = dpool.tile([rows, D], F16)
            a2a_out = dpool.tile([rows, D], F16)

            # resident small tensors (one batched DMA each)
            b1t = sbuf.tile([128, 2 * MF], F32, tag="b1t", name="b1t", bufs=1)
            sb1t = sbuf.tile([128, 2 * MF], F32, tag="sb1t", name="sb1t",
                             bufs=1)
            cwt = sbuf.tile([128, NT], F32, tag="cwt", name="cwt", bufs=1)
            sct = sbuf.tile([128, NT], I32, tag="sct", name="sct", bufs=1)
            g0t = sbuf.tile([128, NTS], I32, tag="g0t", name="g0t", bufs=1)
            g1t = sbuf.tile([128, NTS], I32, tag="g1t", name="g1t", bufs=1)
            nc.sync.dma_start(out=b1t[:], in_=b1[:])
            nc.sync.dma_start(out=sb1t[:], in_=sb1[:])
            b2t = sbuf.tile([128, D], F32, tag="b2t", name="b2t", bufs=1)
            sb2t = sbuf.tile([128, D], F32, tag="sb2t", name="sb2t", bufs=1)
            nc.gpsimd.dma_start(out=b2t[:], in_=b2.to_broadcast([128, D]))
            nc.gpsimd.dma_start(out=sb2t[:], in_=sb2.to_broadcast([128, D]))

            # DMA-queue order mirrors first-use order so the PE can start
            # ~25us in: fi0 weights + the first stage-1 chunk's x columns
            # land first, the x remainder follows
            c0 = chunks_r[0][1]
            w1a0 = sbuf.tile([128, KD, 128], F16, tag="w1a", name="w1a",
                             bufs=6)
            w1b0 = sbuf.tile([128, KD, 128], F16, tag="w1b", name="w1b",
                             bufs=6)
            nc.sync.dma_start(out=w1a0[:], in_=w1[0])
            xk = []
            for k in range(KD):
                xt = sbuf.tile([128, c_alloc], F16, tag=f"xk{k}",
                               name=f"xk{k}", bufs=1)
                nc.sync.dma_start(out=xt[:, :c0], in_=xg[k][:, :c0])
                xk.append(xt)
            nc.sync.dma_start(out=w1b0[:], in_=w1[MF])
            for k in range(KD):
                nc.sync.dma_start(out=xk[k][:, c0:], in_=xg[k][:, c0:])

            # stage-1 output tiles; zero the pad columns once so stage-2
            # matmuls on the last token tile read finite values
            g_tiles = []
            for fi in range(PERS):
                g_t = sbuf.tile([128, c_alloc], F16, tag=f"g{fi}",
                                name=f"g{fi}", bufs=1)
                if c_str < c_alloc:
                    nc.vector.memset(g_t[:, c_str:c_alloc], 0.0)
                g_tiles.append(g_t)

            y_tiles = [sbuf.tile([128, D], F16, tag=f"y{t}", name=f"y{t}",
                                 bufs=1) for t in range(NT)]

            def ffn(xk_tiles, w1d, w2d, b1t_, bias2_t, n_t, chunks,
                    w1_pre=None, final=None):
                """One SwiGLU FFN pass; writes y_tiles[0..n_t-1] (fp16,
                bias2 folded in). final=(r_tiles, out_ap) additionally emits
                the per-tile combine + output DMA inline with the last
                slab's stage-2."""
                for q in range(NSLAB):
                    w2t = sbuf.tile([128, PERS, D], F16, tag="w2",
                                    name="w2", bufs=2)
                    for fi in range(PERS):
                        mp = q * PERS + fi
                        if fi == 0 and q == 0 and w1_pre is not None:
                            w1a, w1b = w1_pre
                        else:
                            w1a = sbuf.tile([128, KD, 128], F16, tag="w1a",
                                            name="w1a", bufs=6)
                            w1b = sbuf.tile([128, KD, 128], F16, tag="w1b",
                                            name="w1b", bufs=6)
                            nc.sync.dma_start(out=w1a[:], in_=w1d[mp])
                            nc.sync.dma_start(out=w1b[:], in_=w1d[mp + MF])
                        if fi == 1:
                            nc.sync.dma_start(out=w2t[:], in_=w2d[q])
                        g_t = g_tiles[fi]
                        for cs, cw in chunks:
                            ps_a = psum.tile([128, 512], F32, space="PSUM",
                                             tag="ps_a", name="ps_a", bufs=3)
                            ps_b = psum.tile([128, 512], F32, space="PSUM",
                                             tag="ps_b", name="ps_b", bufs=3)
                            for k in range(KD):
                                nc.tensor.matmul(out=ps_a[:, :cw],
                                                 lhsT=w1a[:, k, :],
                                                 rhs=xk_tiles[k][:, cs:cs + cw],
                                                 start=(k == 0),
                                                 stop=(k == KD - 1))
                            for k in range(KD):
                                nc.tensor.matmul(out=ps_b[:, :cw],
                                                 lhsT=w1b[:, k, :],
                                                 rhs=xk_tiles[k][:, cs:cs + cw],
                                                 start=(k == 0),
                                                 stop=(k == KD - 1))
                            t_a = sbuf.tile([128, 512], F16, tag="t_a",
                                            name="t_a", bufs=2)
                            t_b = sbuf.tile([128, 512], F16, tag="t_b",
                                            name="t_b", bufs=2)
                            nc.scalar.activation(
                                t_a[:, :cw], ps_a[:, :cw],
                                mybir.ActivationFunctionType.Silu,
                                bias=b1t_[:, mp:mp + 1])
                            nc.scalar.activation(
                                t_b[:, :cw], ps_b[:, :cw],
                                mybir.ActivationFunctionType.Identity,
                                bias=b1t_[:, mp + MF:mp + MF + 1])
                            nc.vector.tensor_mul(g_t[:, cs:cs + cw],
                                                 t_a[:, :cw], t_b[:, :cw])
                    # stage-2 partial: y (+)= g_slab.T @ w2_slab
                    for t in range(n_t):
                        ts = slice(t * 128, (t + 1) * 128)
                        for dd in range(D // 512):
                            ds = slice(dd * 512, (dd + 1) * 512)
                            ps_y = psum.tile([128, 512], F32, space="PSUM",
                                             tag="ps_y", name="ps_y", bufs=2)
                            for fi in range(PERS):
                                nc.tensor.matmul(out=ps_y[:],
                                                 lhsT=g_tiles[fi][:, ts],
                                                 rhs=w2t[:, fi, ds],
                                                 start=(fi == 0),
                                                 stop=(fi == PERS - 1))
                            yt = y_tiles[t]
                            if q == 0:
                                nc.vector.tensor_add(yt[:, ds], ps_y[:],
                                                     bias2_t[:, ds])
                            else:
                                nc.vector.tensor_add(yt[:, ds], yt[:, ds],
                                                     ps_y[:])
                            if q == NSLAB - 1 and final is not None:
                                r_t, out_ap = final
                                if dd == 0:
                                    yo = sbuf.tile([128, D], F32, tag="yo",
                                                   name="yo", bufs=2)
                                nc.vector.tensor_add(yo[:, ds], yt[:, ds],
                                                     r_t[t][:, ds])
                        if q == NSLAB - 1 and final is not None:
                            nc.sync.dma_start(
                                out=final[1][t * 128:(t + 1) * 128, :],
                                in_=yo[:])

            # ---------------- routed expert (single pass) ------------------
            ffn(xk, w1, w2, b1t, b2t, NT, chunks_r, w1_pre=(w1a0, w1b0))
            nc.sync.dma_start(out=cwt[:], in_=cwd[:])
            nc.sync.dma_start(out=sct[:], in_=scat[:])
            nc.sync.dma_start(out=g0t[:], in_=g0i[:])
            nc.sync.dma_start(out=g1t[:], in_=g1i[:])

            # finalize: scale by combine weight, scatter into a2a_in
            for t in range(NT):
                yh = sbuf.tile([128, D], F16, tag="yh", name="yh", bufs=1)
                nc.vector.tensor_scalar_mul(yh[:], y_tiles[t][:],
                                            cwt[:, t:t + 1])
                nc.gpsimd.indirect_dma_start(
                    out=a2a_in[:],
                    out_offset=bass.IndirectOffsetOnAxis(ap=sct[:, t:t + 1],
                                                         axis=0),
                    in_=yh[:],
                    in_offset=None,
                    bounds_check=rows - 1,
                    oob_is_err=False,
                )
            nc.gpsimd.collective_compute(
                "AllToAll",
                mybir.AluOpType.bypass,
                replica_groups=[list(range(NCORES))],
                ins=[a2a_in[:].opt()],
                outs=[a2a_out[:].opt()],
            )

            # combine gathers: r[t] = contrib0 + contrib1 (accumulated in DMA)
            r_tiles = []
            for t in range(NTS):
                rt = sbuf.tile([128, D], F16, tag=f"r{t}", name=f"r{t}",
                               bufs=1)
                nc.gpsimd.indirect_dma_start(
                    out=rt[:], out_offset=None, in_=a2a_out[:],
                    in_offset=bass.IndirectOffsetOnAxis(ap=g0t[:, t:t + 1],
                                                        axis=0))
                nc.gpsimd.indirect_dma_start(
                    out=rt[:], out_offset=None, in_=a2a_out[:],
                    in_offset=bass.IndirectOffsetOnAxis(ap=g1t[:, t:t + 1],
                                                        axis=0),
                    compute_op=mybir.AluOpType.add)
                r_tiles.append(rt)

            # ---------------- shared expert on owned tokens (overlaps) -----
            xsk = []
            for k in range(KD):
                xt = sbuf.tile([128, c_alloc], F16, tag=f"xk{k}",
                               name=f"xk{k}", bufs=1)
                nc.sync.dma_start(out=xt[:, :TO], in_=xs[k])
                xsk.append(xt)
            ffn(xsk, sw1, sw2, sb1t, sb2t, NTS, chunks_s,
                final=(r_tiles, out))

    nc.compile()
    _nc_cache[key] = nc
    return nc


def _route(x, gate_w, gate_b):
    """Host gate: softmax top-2 (float64 for stable ordering)."""
    logits = (x.astype(np.float64) @ gate_w.astype(np.float64)
              + gate_b.astype(np.float64))
    m = logits.max(axis=-1, keepdims=True)
    p = np.exp(logits - m)
    p /= p.sum(axis=-1, keepdims=True)
    order = np.argsort(-p, axis=-1)
    idx = order[:, :TOPK]                      # [T, 2]
    wts = np.take_along_axis(p, idx, axis=-1)  # [T, 2]
    return idx, wts.astype(np.float32)


def kernel(x, gate_w, gate_b, shared_w1, shared_b1, shared_w2, shared_b2,
           routed_w1, routed_b1, routed_w2, routed_b2):
    x = np.asarray(x, dtype=np.float32)
    topk_idx, topk_w = _route(x, np.asarray(gate_w), np.asarray(gate_b))

    owner = np.arange(T) // TO                 # owning core per token

    # per-expert dispatch lists, ordered by (owner, token)
    tok_lists, wt_lists, pos_lists = [], [], []
    p_cap = 0
    for e in range(E):
        sel = (topk_idx == e)                  # [T, 2]
        tsel = np.nonzero(sel.any(axis=1))[0]  # ascending => owner-sorted
        k_of = sel[tsel, 1].astype(np.int64)   # slot (experts distinct)
        w_of = topk_w[tsel, :][np.arange(len(tsel)), k_of]
        own = owner[tsel]
        pos = np.zeros(len(tsel), np.int64)
        for o in range(NCORES):
            mask = own == o
            n = int(mask.sum())
            pos[mask] = np.arange(n)
            p_cap = max(p_cap, n)
        tok_lists.append(tsel)
        wt_lists.append(w_of)
        pos_lists.append(pos)

    c_max = max(len(tl) for tl in tok_lists)
    NT = -(-c_max // 128)
    c_alloc = NT * 128
    rows = NCORES * p_cap

    nc = _build(c_max, p_cap)

    # host-side layouts (fp16 compute dtype)
    w1r = np.asarray(routed_w1, np.float16)              # [E, D, 2F]
    w2r = np.asarray(routed_w2, np.float16)              # [E, F, D]
    sw1r = np.asarray(shared_w1, np.float16)[0]          # [D, 2F]
    sw2r = np.asarray(shared_w2, np.float16)[0]          # [F, D]
    xr = x.astype(np.float16)                            # [T, D]

    def tile_w1(w):                # [D,2F] -> [mp=64, p=128, k=8, c=128]
        return np.ascontiguousarray(
            w.reshape(KD, 128, 2 * MF, 128).transpose(2, 1, 0, 3))

    def tile_w2(w):                # [F,D] -> [q=4, p=128, fj=8, d=1024]
        return np.ascontiguousarray(
            w.reshape(NSLAB, PERS, 128, D).transpose(0, 2, 1, 3))

    def col_bias(b):               # [2F] -> [128, 64]
        return np.ascontiguousarray(
            np.asarray(b, np.float32).reshape(2 * MF, 128).T)

    sw1_t = tile_w1(sw1r)
    sw2_t = tile_w2(sw2r)
    sb1_t = col_bias(np.asarray(shared_b1)[0])

    # a2a_out row for each (token, slot): src_expert * p_cap + pos
    slot_rows = np.zeros((T, TOPK), np.int64)
    for e in range(E):
        toks = tok_lists[e]
        sel = (topk_idx[toks] == e)
        k_of = sel[:, 1].astype(np.int64)
        slot_rows[toks, k_of] = e * p_cap + pos_lists[e]

    in_maps = []
    for c in range(NCORES):
        e = c
        toks = tok_lists[e]
        wts = wt_lists[e]
        ce = len(toks)

        xg_a = np.zeros((KD, 128, c_alloc), np.float16)
        xg_a[:, :, :ce] = xr[toks].T.reshape(KD, 128, ce)

        cw_a = np.zeros((NT * 128,), np.float32)
        cw_a[:ce] = wts
        cw_t = np.ascontiguousarray(cw_a.reshape(NT, 128).T)

        scat_a = np.full((NT * 128,), 2**31 - 1, np.int32)
        scat_a[:ce] = (owner[toks] * p_cap + pos_lists[e]).astype(np.int32)
        scat_t = np.ascontiguousarray(scat_a.reshape(NT, 128).T)

        xs_a = np.ascontiguousarray(
            xr[c * TO:(c + 1) * TO].T.reshape(KD, 128, TO))

        g0 = slot_rows[c * TO:(c + 1) * TO, 0].astype(np.int32)
        g1 = slot_rows[c * TO:(c + 1) * TO, 1].astype(np.int32)
        g0_t = np.ascontiguousarray(g0.reshape(TO // 128, 128).T)
        g1_t = np.ascontiguousarray(g1.reshape(TO // 128, 128).T)

        in_maps.append({
            "xg": xg_a, "xs": xs_a,
            "w1": tile_w1(w1r[e]), "w2": tile_w2(w2r[e]),
            "sw1": sw1_t, "sw2": sw2_t,
            "b1": col_bias(np.asarray(routed_b1)[e]),
            "sb1": sb1_t,
            "b2": np.asarray(routed_b2, np.float32)[e].reshape(1, D).copy(),
            "sb2": np.asarray(shared_b2, np.float32)[0].reshape(1, D).copy(),
            "cw": cw_t, "scat": scat_t, "g0i": g0_t, "g1i": g1_t,
        })

    res = run_bass_kernel_spmd(nc, in_maps, list(range(NCORES)))
    return np.concatenate([res.results[c]["out"] for c in range(NCORES)],
                          axis=0)


# revision 11
# speedup vs baseline: 1.0933x; 1.0165x over previous
"""MoE (top-2 of 8 experts + 1 shared expert, SwiGLU FFN) on 8 TRN2 NeuronCores.

Strategy (expert-parallel with pairwise token-split load balancing):
  - Host computes the gate (softmax top-2) and the dispatch maps.
  - Experts are paired heaviest-with-lightest; each core of a pair processes
    HALF of each paired expert's tokens as two sequential FFN segments
    (seg1 = the heavy expert's half, seg2 = the light expert's half), using
    that expert's full weights. This caps the compile-time stream lengths at
    s1 = ceil(max_e c_e / 2), s2 = ceil(max-light c_e / 2) instead of the
    unsplit max c_e — less padding than plain 1-expert-per-core when expert
    loads are imbalanced, with no change to the collective structure (every
    token's contribution is computed wholly on one core).
  - Routed outputs are scaled by the gate weight and scattered into a single
    AllToAll dispatch buffer (p_cap rows per (src,dst) block, exact).
  - The AllToAll overlaps the shared-expert FFN (each core runs its own T/8
    token slice). Combine: out[t] = shared(t) + r(t), where r(t) sums the two
    routed contributions inside the indirect gather DMA (compute_op=add).
  - Host concatenates the 8 [T/8, D] output slices.

Perf notes:
  - The PE clock here is GPIO-power-capped at 13/16 x 2.4 = 1.95 GHz; the
    kernel is tensor-bound, so the levers are cycle count and keeping the
    PE fed.
  - All DMAs are batched into large single transfers (w1: one 256KB load per
    f-chunk via host retiling, w2: one 2MB load per slab, index/weight
    vectors: one load each); first loads are ordered in exact first-use order.
  - During a collective, HWDGE model DMA starves; w1 prefetch depth (bufs=8)
    rides out the single ~50us AllToAll window.
  - Stage-2 accumulators are fp16 (SBUF budget); bias adds fold into the
    stage-2 PSUM drain; final combine+store interleaves with shared stage-2.

Compute dtype fp16 (~8e-4 rel err vs fp32 reference, threshold 2e-2);
PSUM accumulation fp32.
"""
import contextlib

import numpy as np

import concourse.bass as bass
import concourse.tile as tile
from concourse import bacc, mybir
from concourse.bass_utils import run_bass_kernel_spmd

# problem shape (hardcoded per contract)
T = 8192
D = 1024
F = 4096
E = 8
TOPK = 2
NCORES = 8
TO = T // NCORES          # tokens owned per core
KD = D // 128             # 8 contraction chunks for stage 1
MF = 2 * F // 128 // 2    # 32 a-chunks (b-chunks at +MF)
NSLAB = 4
PERS = MF // NSLAB        # 8 f-chunks per slab

F32 = mybir.dt.float32
F16 = mybir.dt.float16
I32 = mybir.dt.int32

_nc_cache: dict[tuple, object] = {}


def _chunk_slices(c_len):
    """Moving-dim chunks of <=512, each >=256 where possible."""
    out = []
    pos = 0
    rem = c_len
    while rem > 0:
        if rem > 512:
            w = 512 if rem - 512 >= 256 else 384
        else:
            w = rem
        out.append((pos, w))
        pos += w
        rem -= w
    return out


def _build(s1, s2, p_cap):
    key = (s1, s2, p_cap)
    if key in _nc_cache:
        return _nc_cache[key]

    NT1 = -(-s1 // 128)
    NT2 = -(-s2 // 128)
    A1 = NT1 * 128                    # seg2's column base in xg
    NT = NT1 + NT2                    # routed token tiles total
    c_alloc = A1 + NT2 * 128
    g_alloc = max(A1, NT2 * 128, TO)
    NTS = TO // 128                   # shared token tiles (8)
    rows = NCORES * p_cap
    chunks_1 = _chunk_slices(s1)
    chunks_2 = _chunk_slices(s2)
    chunks_s = _chunk_slices(TO)

    nc = bacc.Bacc("TRN2", target_bir_lowering=False, debug=False,
                   num_devices=NCORES)

    def din(name, shape, dt):
        return nc.dram_tensor(name, shape, dt, kind="ExternalInput").ap()

    xg = din("xg", [KD, 128, c_alloc], F16)        # gathered tokens^T
    xs = din("xs", [KD, 128, TO], F16)             # owned tokens^T
    w1A = din("w1A", [2 * MF, 128, KD, 128], F16)  # [mp, p, k, c]
    w2A = din("w2A", [NSLAB, 128, PERS, D], F16)   # [q, p, fj, d]
    w1B = din("w1B", [2 * MF, 128, KD, 128], F16)
    w2B = din("w2B", [NSLAB, 128, PERS, D], F16)
    sw1 = din("sw1", [2 * MF, 128, KD, 128], F16)
    sw2 = din("sw2", [NSLAB, 128, PERS, D], F16)
    b1A = din("b1A", [128, 2 * MF], F32)           # col m = chunk-m bias
    b1B = din("b1B", [128, 2 * MF], F32)
    sb1 = din("sb1", [128, 2 * MF], F32)
    b2A = din("b2A", [1, D], F32)
    b2B = din("b2B", [1, D], F32)
    sb2 = din("sb2", [1, D], F32)
    cwd = din("cw", [128, NT], F32)                # combine weights (col t)
    scat = din("scat", [128, NT], I32)             # scatter row in a2a_in
    g0i = din("g0i", [128, NTS], I32)              # gather rows in a2a_out
    g1i = din("g1i", [128, NTS], I32)
    out = nc.dram_tensor("out", [TO, D], F32, kind="ExternalOutput").ap()

    with tile.TileContext(nc) as tc:
        with contextlib.ExitStack() as ctx:
            sbuf = ctx.enter_context(tc.tile_pool(name="sbuf", bufs=1))
            psum = ctx.enter_context(tc.tile_pool(name="psum", bufs=2,
                                                  space="PSUM"))
            dpool = ctx.enter_context(tc.tile_pool(name="dram", bufs=1,
                                                   space="DRAM"))

            a2a_in = dpool.tile([rows, D], F16)
            a2a_out = dpool.tile([rows, D], F16)

            # resident small tensors (one batched DMA each)
            b1At = sbuf.tile([128, 2 * MF], F32, tag="b1A", name="b1A",
                             bufs=1)
            b1Bt = sbuf.tile([128, 2 * MF], F32, tag="b1B", name="b1B",
                             bufs=1)
            sb1t = sbuf.tile([128, 2 * MF], F32, tag="sb1t", name="sb1t",
                             bufs=1)
            cwt = sbuf.tile([128, NT], F32, tag="cwt", name="cwt", bufs=1)
            sct = sbuf.tile([128, NT], I32, tag="sct", name="sct", bufs=1)
            g0t = sbuf.tile([128, NTS], I32, tag="g0t", name="g0t", bufs=1)
            g1t = sbuf.tile([128, NTS], I32, tag="g1t", name="g1t", bufs=1)
            nc.sync.dma_start(out=b1At[:], in_=b1A[:])
            b2At = sbuf.tile([128, D], F32, tag="b2A", name="b2A", bufs=1)
            b2Bt = sbuf.tile([128, D], F32, tag="b2B", name="b2B", bufs=1)
            sb2t = sbuf.tile([128, D], F32, tag="sb2t", name="sb2t", bufs=1)
            nc.gpsimd.dma_start(out=b2At[:], in_=b2A.to_broadcast([128, D]))
            nc.gpsimd.dma_start(out=b2Bt[:], in_=b2B.to_broadcast([128, D]))
            nc.gpsimd.dma_start(out=sb2t[:], in_=sb2.to_broadcast([128, D]))

            # DMA-queue order mirrors first-use order: seg1 fi0 weights +
            # the first stage-1 chunk's x columns land first
            c0 = chunks_1[0][1]
            w1a0 = sbuf.tile([128, KD, 128], F16, tag="w1a", name="w1a",
                             bufs=8)
            w1b0 = sbuf.tile([128, KD, 128], F16, tag="w1b", name="w1b",
                             bufs=8)
            nc.sync.dma_start(out=w1a0[:], in_=w1A[0])
            xk = []
            for k in range(KD):
                xt = sbuf.tile([128, c_alloc], F16, tag=f"xk{k}",
                               name=f"xk{k}", bufs=1)
                nc.sync.dma_start(out=xt[:, :c0], in_=xg[k][:, :c0])
                xk.append(xt)
            nc.sync.dma_start(out=w1b0[:], in_=w1A[MF])
            for k in range(KD):
                nc.sync.dma_start(out=xk[k][:, c0:], in_=xg[k][:, c0:])
            nc.sync.dma_start(out=b1Bt[:], in_=b1B[:])
            nc.sync.dma_start(out=sb1t[:], in_=sb1[:])

            g_tiles = [sbuf.tile([128, g_alloc], F16, tag=f"g{fi}",
                                 name=f"g{fi}", bufs=1)
                       for fi in range(PERS)]

            y_tiles = [sbuf.tile([128, D], F16, tag=f"y{t}", name=f"y{t}",
                                 bufs=1) for t in range(NT)]

            def g_pad(c_len, n_t):
                """Zero g pad columns so stage-2 reads finite values."""
                if c_len < n_t * 128:
                    for g_t in g_tiles:
                        nc.vector.memset(g_t[:, c_len:n_t * 128], 0.0)

            def ffn(w1d, w2d, b1t_, bias2_t, n_t, chunks, col0=0, y_off=0,
                    w1_pre=None, final=None):
                """One SwiGLU FFN pass over xk columns [col0, col0+len);
                writes y_tiles[y_off..y_off+n_t-1] (fp16, bias2 folded in).
                final=(r_tiles, out_ap) additionally emits the per-tile
                combine + output DMA inline with the last slab's stage-2."""
                for q in range(NSLAB):
                    w2t = sbuf.tile([128, PERS, D], F16, tag="w2",
                                    name="w2", bufs=2)
                    for fi in range(PERS):
                        mp = q * PERS + fi
                        if fi == 0 and q == 0 and w1_pre is not None:
                            w1a, w1b = w1_pre
                        else:
                            w1a = sbuf.tile([128, KD, 128], F16, tag="w1a",
                                            name="w1a", bufs=8)
                            w1b = sbuf.tile([128, KD, 128], F16, tag="w1b",
                                            name="w1b", bufs=8)
                            nc.sync.dma_start(out=w1a[:], in_=w1d[mp])
                            nc.sync.dma_start(out=w1b[:], in_=w1d[mp + MF])
                        if fi == 1:
                            nc.sync.dma_start(out=w2t[:], in_=w2d[q])
                        g_t = g_tiles[fi]
                        for cs, cw in chunks:
                            ps_a = psum.tile([128, 512], F32, space="PSUM",
                                             tag="ps_a", name="ps_a", bufs=3)
                            ps_b = psum.tile([128, 512], F32, space="PSUM",
                                             tag="ps_b", name="ps_b", bufs=3)
                            xs_ = slice(col0 + cs, col0 + cs + cw)
                            for k in range(KD):
                                nc.tensor.matmul(out=ps_a[:, :cw],
                                                 lhsT=w1a[:, k, :],
                                                 rhs=xk[k][:, xs_],
                                                 start=(k == 0),
                                                 stop=(k == KD - 1))
                            for k in range(KD):
                                nc.tensor.matmul(out=ps_b[:, :cw],
                                                 lhsT=w1b[:, k, :],
                                                 rhs=xk[k][:, xs_],
                                                 start=(k == 0),
                                                 stop=(k == KD - 1))
                            t_a = sbuf.tile([128, 512], F16, tag="t_a",
                                            name="t_a", bufs=2)
                            t_b = sbuf.tile([128, 512], F16, tag="t_b",
                                            name="t_b", bufs=2)
                            nc.scalar.activation(
                                t_a[:, :cw], ps_a[:, :cw],
                                mybir.ActivationFunctionType.Silu,
                                bias=b1t_[:, mp:mp + 1])
                            nc.scalar.activation(
                                t_b[:, :cw], ps_b[:, :cw],
                                mybir.ActivationFunctionType.Identity,
                                bias=b1t_[:, mp + MF:mp + MF + 1])
                            nc.vector.tensor_mul(g_t[:, cs:cs + cw],
                                                 t_a[:, :cw], t_b[:, :cw])
                    # stage-2 partial: y (+)= g_slab.T @ w2_slab
                    for t in range(n_t):
                        ts = slice(t * 128, (t + 1) * 128)
                        for dd in range(D // 512):
                            ds = slice(dd * 512, (dd + 1) * 512)
                            ps_y = psum.tile([128, 512], F32, space="PSUM",
                                             tag="ps_y", name="ps_y", bufs=2)
                            for fi in range(PERS):
                                nc.tensor.matmul(out=ps_y[:],
                                                 lhsT=g_tiles[fi][:, ts],
                                                 rhs=w2t[:, fi, ds],
                                                 start=(fi == 0),
                                                 stop=(fi == PERS - 1))
                            yt = y_tiles[y_off + t]
                            if q == 0:
                                nc.vector.tensor_add(yt[:, ds], ps_y[:],
                                                     bias2_t[:, ds])
                            else:
                                nc.vector.tensor_add(yt[:, ds], yt[:, ds],
                                                     ps_y[:])
                            if q == NSLAB - 1 and final is not None:
                                r_t, out_ap = final
                                if dd == 0:
                                    yo = sbuf.tile([128, D], F32, tag="yo",
                                                   name="yo", bufs=2)
                                nc.vector.tensor_add(yo[:, ds], yt[:, ds],
                                                     r_t[t][:, ds])
                        if q == NSLAB - 1 and final is not None:
                            nc.sync.dma_start(
                                out=final[1][t * 128:(t + 1) * 128, :],
                                in_=yo[:])

            # ---------------- routed segments (seg1 then seg2) -------------
            g_pad(s1, NT1)
            ffn(w1A, w2A, b1At, b2At, NT1, chunks_1, col0=0, y_off=0,
                w1_pre=(w1a0, w1b0))
            g_pad(s2, NT2)
            ffn(w1B, w2B, b1Bt, b2Bt, NT2, chunks_2, col0=A1, y_off=NT1)
            nc.sync.dma_start(out=cwt[:], in_=cwd[:])
            nc.sync.dma_start(out=sct[:], in_=scat[:])
            nc.sync.dma_start(out=g0t[:], in_=g0i[:])
            nc.sync.dma_start(out=g1t[:], in_=g1i[:])

            # finalize: scale by combine weight, scatter into a2a_in
            for t in range(NT):
                yh = sbuf.tile([128, D], F16, tag="yh", name="yh", bufs=1)
                nc.vector.tensor_scalar_mul(yh[:], y_tiles[t][:],
                                            cwt[:, t:t + 1])
                nc.gpsimd.indirect_dma_start(
                    out=a2a_in[:],
                    out_offset=bass.IndirectOffsetOnAxis(ap=sct[:, t:t + 1],
                                                         axis=0),
                    in_=yh[:],
                    in_offset=None,
                    bounds_check=rows - 1,
                    oob_is_err=False,
                )
            nc.gpsimd.collective_compute(
                "AllToAll",
                mybir.AluOpType.bypass,
                replica_groups=[list(range(NCORES))],
                ins=[a2a_in[:].opt()],
                outs=[a2a_out[:].opt()],
            )

            # combine gathers: r[t] = contrib0 + contrib1 (accumulated in DMA)
            r_tiles = []
            for t in range(NTS):
                rt = sbuf.tile([128, D], F16, tag=f"r{t}", name=f"r{t}",
                               bufs=1)
                nc.gpsimd.indirect_dma_start(
                    out=rt[:], out_offset=None, in_=a2a_out[:],
                    in_offset=bass.IndirectOffsetOnAxis(ap=g0t[:, t:t + 1],
                                                        axis=0))
                nc.gpsimd.indirect_dma_start(
                    out=rt[:], out_offset=None, in_=a2a_out[:],
                    in_offset=bass.IndirectOffsetOnAxis(ap=g1t[:, t:t + 1],
                                                        axis=0),
                    compute_op=mybir.AluOpType.add)
                r_tiles.append(rt)

            # ---------------- shared expert on owned tokens (overlaps) -----
            for k in range(KD):
                nc.sync.dma_start(out=xk[k][:, :TO], in_=xs[k])
            ffn(sw1, sw2, sb1t, sb2t, NTS, chunks_s,
                final=(r_tiles, out))

    nc.compile()
    _nc_cache[key] = nc
    return nc


def _route(x, gate_w, gate_b):
    """Host gate: softmax top-2 (float64 for stable ordering)."""
    logits = (x.astype(np.float64) @ gate_w.astype(np.float64)
              + gate_b.astype(np.float64))
    m = logits.max(axis=-1, keepdims=True)
    p = np.exp(logits - m)
    p /= p.sum(axis=-1, keepdims=True)
    order = np.argsort(-p, axis=-1)
    idx = order[:, :TOPK]                      # [T, 2]
    wts = np.take_along_axis(p, idx, axis=-1)  # [T, 2]
    return idx, wts.astype(np.float32)


def kernel(x, gate_w, gate_b, shared_w1, shared_b1, shared_w2, shared_b2,
           routed_w1, routed_b1, routed_w2, routed_b2):
    x = np.asarray(x, dtype=np.float32)
    topk_idx, topk_w = _route(x, np.asarray(gate_w), np.asarray(gate_b))

    owner = np.arange(T) // TO                 # owning core per token

    # per-expert dispatch lists (ascending token order => owner-sorted)
    tok_lists, wt_lists = [], []
    for e in range(E):
        sel = (topk_idx == e)                  # [T, 2]
        tsel = np.nonzero(sel.any(axis=1))[0]
        k_of = sel[tsel, 1].astype(np.int64)   # slot (experts distinct)
        w_of = topk_w[tsel, :][np.arange(len(tsel)), k_of]
        tok_lists.append(tsel)
        wt_lists.append(w_of)

    counts = np.array([len(t) for t in tok_lists])
    # pair heaviest with lightest; each pair-core gets half of each expert
    order_desc = np.argsort(-counts)
    bigs = order_desc[:4]
    smalls = order_desc[4:][::-1]              # lightest first
    s1 = int(-(-counts[bigs].max() // 2))
    s2 = int(-(-counts[smalls].max() // 2))
    NT1 = -(-s1 // 128)
    NT2 = -(-s2 // 128)
    A1 = NT1 * 128
    NT = NT1 + NT2
    c_alloc = A1 + NT2 * 128

    # per-core token lists: seg1 = half of big expert, seg2 = half of small
    core_exp = []                              # (expA, sliceA, expB, sliceB)
    for p in range(4):
        a, b = int(bigs[p]), int(smalls[p])
        ca, cb = counts[a], counts[b]
        xa, yb = -(-ca // 2), -(-cb // 2)
        core_exp.append((a, slice(0, xa), b, slice(0, yb)))
        core_exp.append((a, slice(xa, ca), b, slice(yb, cb)))

    core_toks, core_wts, core_seg = [], [], []
    for c in range(NCORES):
        a, sa, b, sb_ = core_exp[c]
        ta, tb = tok_lists[a][sa], tok_lists[b][sb_]
        wa, wb = wt_lists[a][sa], wt_lists[b][sb_]
        core_toks.append((ta, tb))
        core_wts.append((wa, wb))
        core_seg.append((a, b))

    # positions within (computing core -> owner) blocks; p_cap exact
    p_cap = 0
    core_pos = []
    for c in range(NCORES):
        ta, tb = core_toks[c]
        allt = np.concatenate([ta, tb])
        own = owner[allt]
        pos = np.zeros(len(allt), np.int64)
        for o in range(NCORES):
            mask = own == o
            n = int(mask.sum())
            pos[mask] = np.arange(n)
            p_cap = max(p_cap, n)
        core_pos.append(pos)
    rows = NCORES * p_cap

    nc = _build(s1, s2, p_cap)

    # host-side layouts (fp16 compute dtype)
    w1r = np.asarray(routed_w1, np.float16)              # [E, D, 2F]
    w2r = np.asarray(routed_w2, np.float16)              # [E, F, D]
    sw1r = np.asarray(shared_w1, np.float16)[0]          # [D, 2F]
    sw2r = np.asarray(shared_w2, np.float16)[0]          # [F, D]
    b1r = np.asarray(routed_b1, np.float32)
    b2r = np.asarray(routed_b2, np.float32)
    xr = x.astype(np.float16)                            # [T, D]

    def tile_w1(w):                # [D,2F] -> [mp=64, p=128, k=8, c=128]
        return np.ascontiguousarray(
            w.reshape(KD, 128, 2 * MF, 128).transpose(2, 1, 0, 3))

    def tile_w2(w):                # [F,D] -> [q=4, p=128, fj=8, d=1024]
        return np.ascontiguousarray(
            w.reshape(NSLAB, PERS, 128, D).transpose(0, 2, 1, 3))

    def col_bias(b):               # [2F] -> [128, 64]
        return np.ascontiguousarray(
            np.asarray(b, np.float32).reshape(2 * MF, 128).T)

    w1_t = {int(e): tile_w1(w1r[e]) for e in range(E)}
    w2_t = {int(e): tile_w2(w2r[e]) for e in range(E)}
    b1_t = {int(e): col_bias(b1r[e]) for e in range(E)}
    sw1_t = tile_w1(sw1r)
    sw2_t = tile_w2(sw2r)
    sb1_t = col_bias(np.asarray(shared_b1)[0])

    # a2a_out row for each (token, slot): computing core * p_cap + pos
    slot_rows = np.zeros((T, TOPK), np.int64)
    for c in range(NCORES):
        ta, tb = core_toks[c]
        allt = np.concatenate([ta, tb])
        exps = np.concatenate([np.full(len(ta), core_seg[c][0]),
                               np.full(len(tb), core_seg[c][1])])
        sel = (topk_idx[allt] == exps[:, None])
        k_of = sel[:, 1].astype(np.int64)
        slot_rows[allt, k_of] = c * p_cap + core_pos[c]

    in_maps = []
    for c in range(NCORES):
        ta, tb = core_toks[c]
        wa, wb = core_wts[c]
        ea, eb = core_seg[c]

        xg_a = np.zeros((KD, 128, c_alloc), np.float16)
        if len(ta):
            xg_a[:, :, :len(ta)] = xr[ta].T.reshape(KD, 128, len(ta))
        if len(tb):
            xg_a[:, :, A1:A1 + len(tb)] = xr[tb].T.reshape(KD, 128, len(tb))

        cw_a = np.zeros((NT * 128,), np.float32)
        scat_a = np.full((NT * 128,), 2**31 - 1, np.int32)
        sc = owner[np.concatenate([ta, tb])] * p_cap + core_pos[c]
        cw_a[:len(ta)] = wa
        cw_a[A1:A1 + len(tb)] = wb
        scat_a[:len(ta)] = sc[:len(ta)].astype(np.int32)
        scat_a[A1:A1 + len(tb)] = sc[len(ta):].astype(np.int32)
        cw_t = np.ascontiguousarray(cw_a.reshape(NT, 128).T)
        scat_t = np.ascontiguousarray(scat_a.reshape(NT, 128).T)

        xs_a = np.ascontiguousarray(
            xr[c * TO:(c + 1) * TO].T.reshape(KD, 128, TO))

        g0 = slot_rows[c * TO:(c + 1) * TO, 0].astype(np.int32)
        g1 = slot_rows[c * TO:(c + 1) * TO, 1].astype(np.int32)
        g0_t = np.ascontiguousarray(g0.reshape(TO // 128, 128).T)
        g1_t = np.ascontiguousarray(g1.reshape(TO // 128, 128).T)

        in_maps.append({
            "xg": xg_a, "xs": xs_a,
            "w1A": w1_t[ea], "w2A": w2_t[ea],
            "w1B": w1_t[eb], "w2B": w2_t[eb],
            "sw1": sw1_t, "sw2": sw2_t,
            "b1A": b1_t[ea], "b1B": b1_t[eb], "sb1": sb1_t,
            "b2A": b2r[ea].reshape(1, D).copy(),
            "b2B": b2r[eb].reshape(1, D).copy(),
            "sb2": np.asarray(shared_b2, np.float32)[0].reshape(1, D).copy(),
            "cw": cw_t, "scat": scat_t, "g0i": g0_t, "g1i": g1_t,
        })

    res = run_bass_kernel_spmd(nc, in_maps, list(range(NCORES)))
    return np.concatenate([res.results[c]["out"] for c in range(NCORES)],
                          axis=0)


# revision 13
# speedup vs baseline: 1.0974x; 1.0037x over previous
"""MoE (top-2 of 8 experts + 1 shared expert, SwiGLU FFN) on 8 TRN2 NeuronCores.

Strategy (expert-parallel with pairwise token-split load balancing):
  - Host computes the gate (softmax top-2) and the dispatch maps.
  - Experts are paired heaviest-with-lightest; each core of a pair processes
    HALF of each paired expert's tokens as two sequential FFN segments
    (seg1 = the heavy expert's half, seg2 = the light expert's half), using
    that expert's full weights. This caps the compile-time stream lengths at
    s1 = ceil(max_e c_e / 2), s2 = ceil(max-light c_e / 2) instead of the
    unsplit max c_e — less padding than plain 1-expert-per-core when expert
    loads are imbalanced, with no change to the collective structure (every
    token's contribution is computed wholly on one core).
  - Routed outputs are scaled by the gate weight and scattered into a single
    AllToAll dispatch buffer (p_cap rows per (src,dst) block, exact).
  - The AllToAll overlaps the shared-expert FFN (each core runs its own T/8
    token slice). Combine: out[t] = shared(t) + r(t), where r(t) sums the two
    routed contributions inside the indirect gather DMA (compute_op=add).
  - Host concatenates the 8 [T/8, D] output slices.

Perf notes:
  - The PE clock here is GPIO-power-capped at 13/16 x 2.4 = 1.95 GHz; the
    kernel is tensor-bound, so the levers are cycle count and keeping the
    PE fed.
  - All DMAs are batched into large single transfers (w1: one 256KB load per
    f-chunk via host retiling, w2: one 2MB load per slab, index/weight
    vectors: one load each); first loads are ordered in exact first-use order.
  - During a collective, HWDGE model DMA starves; w1 prefetch depth (bufs=8)
    rides out the single ~50us AllToAll window.
  - Stage-2 accumulators are fp16 (SBUF budget); bias adds fold into the
    stage-2 PSUM drain; final combine+store interleaves with shared stage-2.

Compute dtype fp16 (~8e-4 rel err vs fp32 reference, threshold 2e-2);
PSUM accumulation fp32.
"""
import contextlib

import numpy as np

import concourse.bass as bass
import concourse.tile as tile
from concourse import bacc, mybir
from concourse.bass_utils import run_bass_kernel_spmd

# problem shape (hardcoded per contract)
T = 8192
D = 1024
F = 4096
E = 8
TOPK = 2
NCORES = 8
TO = T // NCORES          # tokens owned per core
KD = D // 128             # 8 contraction chunks for stage 1
MF = 2 * F // 128 // 2    # 32 a-chunks (b-chunks at +MF)
NSLAB = 4
PERS = MF // NSLAB        # 8 f-chunks per slab

F32 = mybir.dt.float32
F16 = mybir.dt.float16
I32 = mybir.dt.int32

_nc_cache: dict[tuple, object] = {}


def _chunk_slices(c_len):
    """Moving-dim chunks of <=512, each >=256 where possible."""
    out = []
    pos = 0
    rem = c_len
    while rem > 0:
        if rem > 512:
            w = 512 if rem - 512 >= 256 else 384
        else:
            w = rem
        out.append((pos, w))
        pos += w
        rem -= w
    return out


def _build(s1, s2, p_cap):
    key = (s1, s2, p_cap)
    if key in _nc_cache:
        return _nc_cache[key]

    NT1 = -(-s1 // 128)
    NT2 = -(-s2 // 128)
    A1 = NT1 * 128                    # seg2's column base in xg
    NT = NT1 + NT2                    # routed token tiles total
    c_alloc = A1 + NT2 * 128
    g_alloc = max(A1, NT2 * 128, TO)
    NTS = TO // 128                   # shared token tiles (8)
    rows = NCORES * p_cap
    chunks_1 = _chunk_slices(s1)
    chunks_2 = _chunk_slices(s2)
    chunks_s = _chunk_slices(TO)

    nc = bacc.Bacc("TRN2", target_bir_lowering=False, debug=False,
                   num_devices=NCORES)

    def din(name, shape, dt):
        return nc.dram_tensor(name, shape, dt, kind="ExternalInput").ap()

    xg = din("xg", [KD, 128, c_alloc], F16)        # gathered tokens^T
    xs = din("xs", [KD, 128, TO], F16)             # owned tokens^T
    w1A = din("w1A", [2 * MF, 128, KD, 128], F16)  # [mp, p, k, c]
    w2A = din("w2A", [NSLAB, 128, PERS, D], F16)   # [q, p, fj, d]
    w1B = din("w1B", [2 * MF, 128, KD, 128], F16)
    w2B = din("w2B", [NSLAB, 128, PERS, D], F16)
    sw1 = din("sw1", [2 * MF, 128, KD, 128], F16)
    sw2 = din("sw2", [NSLAB, 128, PERS, D], F16)
    b1A = din("b1A", [128, 2 * MF], F32)           # col m = chunk-m bias
    b1B = din("b1B", [128, 2 * MF], F32)
    sb1 = din("sb1", [128, 2 * MF], F32)
    b2A = din("b2A", [1, D], F32)
    b2B = din("b2B", [1, D], F32)
    sb2 = din("sb2", [1, D], F32)
    cwd = din("cw", [128, NT], F32)                # combine weights (col t)
    scat = din("scat", [128, NT], I32)             # scatter row in a2a_in
    g0i = din("g0i", [128, NTS], I32)              # gather rows in a2a_out
    g1i = din("g1i", [128, NTS], I32)
    out = nc.dram_tensor("out", [TO, D], F32, kind="ExternalOutput").ap()

    with tile.TileContext(nc) as tc:
        with contextlib.ExitStack() as ctx:
            sbuf = ctx.enter_context(tc.tile_pool(name="sbuf", bufs=1))
            psum = ctx.enter_context(tc.tile_pool(name="psum", bufs=2,
                                                  space="PSUM"))
            dpool = ctx.enter_context(tc.tile_pool(name="dram", bufs=1,
                                                   space="DRAM"))

            a2a_in = dpool.tile([rows, D], F16)
            a2a_out = dpool.tile([rows, D], F16)

            # resident small tensors (one batched DMA each)
            b1At = sbuf.tile([128, 2 * MF], F32, tag="b1A", name="b1A",
                             bufs=1)
            b1Bt = sbuf.tile([128, 2 * MF], F32, tag="b1B", name="b1B",
                             bufs=1)
            sb1t = sbuf.tile([128, 2 * MF], F32, tag="sb1t", name="sb1t",
                             bufs=1)
            cwt = sbuf.tile([128, NT], F32, tag="cwt", name="cwt", bufs=1)
            sct = sbuf.tile([128, NT], I32, tag="sct", name="sct", bufs=1)
            g0t = sbuf.tile([128, NTS], I32, tag="g0t", name="g0t", bufs=1)
            g1t = sbuf.tile([128, NTS], I32, tag="g1t", name="g1t", bufs=1)
            nc.sync.dma_start(out=b1At[:], in_=b1A[:])
            b2At = sbuf.tile([128, D], F32, tag="b2A", name="b2A", bufs=1)
            b2Bt = sbuf.tile([128, D], F32, tag="b2B", name="b2B", bufs=1)
            sb2t = sbuf.tile([128, D], F32, tag="sb2t", name="sb2t", bufs=1)
            nc.gpsimd.dma_start(out=b2At[:], in_=b2A.to_broadcast([128, D]))
            nc.gpsimd.dma_start(out=b2Bt[:], in_=b2B.to_broadcast([128, D]))
            nc.gpsimd.dma_start(out=sb2t[:], in_=sb2.to_broadcast([128, D]))

            # DMA-queue order mirrors first-use order: seg1 fi0 weights +
            # the first stage-1 chunk's x columns land first
            c0 = chunks_1[0][1]
            w1a0 = sbuf.tile([128, KD, 128], F16, tag="w1a", name="w1a",
                             bufs=8)
            w1b0 = sbuf.tile([128, KD, 128], F16, tag="w1b", name="w1b",
                             bufs=8)
            nc.sync.dma_start(out=w1a0[:], in_=w1A[0])
            xk = []
            for k in range(KD):
                xt = sbuf.tile([128, c_alloc], F16, tag=f"xk{k}",
                               name=f"xk{k}", bufs=1)
                nc.sync.dma_start(out=xt[:, :c0], in_=xg[k][:, :c0])
                xk.append(xt)
            nc.sync.dma_start(out=w1b0[:], in_=w1A[MF])
            for k in range(KD):
                nc.sync.dma_start(out=xk[k][:, c0:], in_=xg[k][:, c0:])
            nc.sync.dma_start(out=b1Bt[:], in_=b1B[:])
            nc.sync.dma_start(out=sb1t[:], in_=sb1[:])

            g_tiles = [sbuf.tile([128, g_alloc], F16, tag=f"g{fi}",
                                 name=f"g{fi}", bufs=1)
                       for fi in range(PERS)]

            y_tiles = [sbuf.tile([128, D], F16, tag=f"y{t}", name=f"y{t}",
                                 bufs=1) for t in range(NT)]

            def g_pad(c_len, n_t):
                """Zero g pad columns so stage-2 reads finite values."""
                if c_len < n_t * 128:
                    for g_t in g_tiles:
                        nc.vector.memset(g_t[:, c_len:n_t * 128], 0.0)

            def ffn(w1d, w2d, b1t_, bias2_t, n_t, chunks, col0=0, y_off=0,
                    w1_pre=None, final=None):
                """One SwiGLU FFN pass over xk columns [col0, col0+len);
                writes y_tiles[y_off..y_off+n_t-1] (fp16, bias2 folded in).
                final=(r_tiles, out_ap) additionally emits the per-tile
                combine + output DMA inline with the last slab's stage-2."""
                for q in range(NSLAB):
                    w2t = sbuf.tile([128, PERS, D], F16, tag="w2",
                                    name="w2", bufs=2)
                    for fi in range(PERS):
                        mp = q * PERS + fi
                        if fi == 0 and q == 0 and w1_pre is not None:
                            w1a, w1b = w1_pre
                        else:
                            w1a = sbuf.tile([128, KD, 128], F16, tag="w1a",
                                            name="w1a", bufs=8)
                            w1b = sbuf.tile([128, KD, 128], F16, tag="w1b",
                                            name="w1b", bufs=8)
                            nc.sync.dma_start(out=w1a[:], in_=w1d[mp])
                            nc.sync.dma_start(out=w1b[:], in_=w1d[mp + MF])
                        if fi == 1:
                            nc.sync.dma_start(out=w2t[:], in_=w2d[q])
                        g_t = g_tiles[fi]
                        for cs, cw in chunks:
                            ps_a = psum.tile([128, 512], F32, space="PSUM",
                                             tag="ps_a", name="ps_a", bufs=3)
                            ps_b = psum.tile([128, 512], F32, space="PSUM",
                                             tag="ps_b", name="ps_b", bufs=3)
                            xs_ = slice(col0 + cs, col0 + cs + cw)
                            for k in range(KD):
                                nc.tensor.matmul(out=ps_a[:, :cw],
                                                 lhsT=w1a[:, k, :],
                                                 rhs=xk[k][:, xs_],
                                                 start=(k == 0),
                                                 stop=(k == KD - 1))
                            for k in range(KD):
                                nc.tensor.matmul(out=ps_b[:, :cw],
                                                 lhsT=w1b[:, k, :],
                                                 rhs=xk[k][:, xs_],
                                                 start=(k == 0),
                                                 stop=(k == KD - 1))
                            t_a = sbuf.tile([128, 512], F16, tag="t_a",
                                            name="t_a", bufs=2)
                            t_b = sbuf.tile([128, 512], F16, tag="t_b",
                                            name="t_b", bufs=2)
                            nc.scalar.activation(
                                t_a[:, :cw], ps_a[:, :cw],
                                mybir.ActivationFunctionType.Silu,
                                bias=b1t_[:, mp:mp + 1])
                            nc.scalar.activation(
                                t_b[:, :cw], ps_b[:, :cw],
                                mybir.ActivationFunctionType.Identity,
                                bias=b1t_[:, mp + MF:mp + MF + 1])
                            nc.vector.tensor_mul(g_t[:, cs:cs + cw],
                                                 t_a[:, :cw], t_b[:, :cw])
                    # stage-2 partial: y (+)= g_slab.T @ w2_slab
                    for t in range(n_t):
                        ts = slice(t * 128, (t + 1) * 128)
                        for dd in range(D // 512):
                            ds = slice(dd * 512, (dd + 1) * 512)
                            ps_y = psum.tile([128, 512], F32, space="PSUM",
                                             tag="ps_y", name="ps_y", bufs=2)
                            for fi in range(PERS):
                                nc.tensor.matmul(out=ps_y[:],
                                                 lhsT=g_tiles[fi][:, ts],
                                                 rhs=w2t[:, fi, ds],
                                                 start=(fi == 0),
                                                 stop=(fi == PERS - 1))
                            yt = y_tiles[y_off + t]
                            if q == 0:
                                nc.vector.tensor_add(yt[:, ds], ps_y[:],
                                                     bias2_t[:, ds])
                            else:
                                nc.vector.tensor_add(yt[:, ds], yt[:, ds],
                                                     ps_y[:])
                            if q == NSLAB - 1 and final is not None:
                                r_t, out_ap = final
                                if dd == 0:
                                    yo = sbuf.tile([128, D], F32, tag="yo",
                                                   name="yo", bufs=2)
                                nc.vector.tensor_add(yo[:, ds], yt[:, ds],
                                                     r_t[t][:, ds])
                        if q == NSLAB - 1 and final is not None:
                            nc.sync.dma_start(
                                out=final[1][t * 128:(t + 1) * 128, :],
                                in_=yo[:])

            # ---------------- routed segments (seg1 then seg2) -------------
            g_pad(s1, NT1)
            ffn(w1A, w2A, b1At, b2At, NT1, chunks_1, col0=0, y_off=0,
                w1_pre=(w1a0, w1b0))
            g_pad(s2, NT2)
            ffn(w1B, w2B, b1Bt, b2Bt, NT2, chunks_2, col0=A1, y_off=NT1)
            nc.sync.dma_start(out=cwt[:], in_=cwd[:])
            nc.sync.dma_start(out=sct[:], in_=scat[:])
            nc.sync.dma_start(out=g0t[:], in_=g0i[:])
            nc.sync.dma_start(out=g1t[:], in_=g1i[:])

            # finalize: scale by combine weight, scatter into a2a_in
            for t in range(NT):
                yh = sbuf.tile([128, D], F16, tag="yh", name="yh", bufs=3)
                nc.vector.tensor_scalar_mul(yh[:], y_tiles[t][:],
                                            cwt[:, t:t + 1])
                nc.gpsimd.indirect_dma_start(
                    out=a2a_in[:],
                    out_offset=bass.IndirectOffsetOnAxis(ap=sct[:, t:t + 1],
                                                         axis=0),
                    in_=yh[:],
                    in_offset=None,
                    bounds_check=rows - 1,
                    oob_is_err=False,
                )
            nc.gpsimd.collective_compute(
                "AllToAll",
                mybir.AluOpType.bypass,
                replica_groups=[list(range(NCORES))],
                ins=[a2a_in[:].opt()],
                outs=[a2a_out[:].opt()],
            )

            # combine gathers: r[t] = contrib0 + contrib1 (accumulated in DMA)
            r_tiles = []
            for t in range(NTS):
                rt = sbuf.tile([128, D], F16, tag=f"r{t}", name=f"r{t}",
                               bufs=1)
                nc.gpsimd.indirect_dma_start(
                    out=rt[:], out_offset=None, in_=a2a_out[:],
                    in_offset=bass.IndirectOffsetOnAxis(ap=g0t[:, t:t + 1],
                                                        axis=0))
                nc.gpsimd.indirect_dma_start(
                    out=rt[:], out_offset=None, in_=a2a_out[:],
                    in_offset=bass.IndirectOffsetOnAxis(ap=g1t[:, t:t + 1],
                                                        axis=0),
                    compute_op=mybir.AluOpType.add)
                r_tiles.append(rt)

            # ---------------- shared expert on owned tokens (overlaps) -----
            for k in range(KD):
                nc.sync.dma_start(out=xk[k][:, :TO], in_=xs[k])
            ffn(sw1, sw2, sb1t, sb2t, NTS, chunks_s,
                final=(r_tiles, out))

    nc.compile()
    _nc_cache[key] = nc
    return nc


def _route(x, gate_w, gate_b):
    """Host gate: softmax top-2 (float64 for stable ordering)."""
    logits = (x.astype(np.float64) @ gate_w.astype(np.float64)
              + gate_b.astype(np.float64))
    m = logits.max(axis=-1, keepdims=True)
    p = np.exp(logits - m)
    p /= p.sum(axis=-1, keepdims=True)
    order = np.argsort(-p, axis=-1)
    idx = order[:, :TOPK]                      # [T, 2]
    wts = np.take_along_axis(p, idx, axis=-1)  # [T, 2]
    return idx, wts.astype(np.float32)


def kernel(x, gate_w, gate_b, shared_w1, shared_b1, shared_w2, shared_b2,
           routed_w1, routed_b1, routed_w2, routed_b2):
    x = np.asarray(x, dtype=np.float32)
    topk_idx, topk_w = _route(x, np.asarray(gate_w), np.asarray(gate_b))

    owner = np.arange(T) // TO                 # owning core per token

    # per-expert dispatch lists (ascending token order => owner-sorted)
    tok_lists, wt_lists = [], []
    for e in range(E):
        sel = (topk_idx == e)                  # [T, 2]
        tsel = np.nonzero(sel.any(axis=1))[0]
        k_of = sel[tsel, 1].astype(np.int64)   # slot (experts distinct)
        w_of = topk_w[tsel, :][np.arange(len(tsel)), k_of]
        tok_lists.append(tsel)
        wt_lists.append(w_of)

    counts = np.array([len(t) for t in tok_lists])
    # pair heaviest with lightest; each pair-core gets half of each expert
    order_desc = np.argsort(-counts)
    bigs = order_desc[:4]
    smalls = order_desc[4:][::-1]              # lightest first
    s1 = int(-(-counts[bigs].max() // 2))
    s2 = int(-(-counts[smalls].max() // 2))
    NT1 = -(-s1 // 128)
    NT2 = -(-s2 // 128)
    A1 = NT1 * 128
    NT = NT1 + NT2
    c_alloc = A1 + NT2 * 128

    # per-core token lists: seg1 = half of big expert, seg2 = half of small.
    # Interleaved split (even/odd positions) so each half spans all owner
    # cores evenly — a contiguous split would concentrate owners and blow
    # up the fixed per-(src,dst) AllToAll block size.
    core_exp = []                              # (expA, selA, expB, selB)
    for p in range(4):
        a, b = int(bigs[p]), int(smalls[p])
        core_exp.append((a, slice(0, None, 2), b, slice(0, None, 2)))
        core_exp.append((a, slice(1, None, 2), b, slice(1, None, 2)))

    core_toks, core_wts, core_seg = [], [], []
    for c in range(NCORES):
        a, sa, b, sb_ = core_exp[c]
        ta, tb = tok_lists[a][sa], tok_lists[b][sb_]
        wa, wb = wt_lists[a][sa], wt_lists[b][sb_]
        core_toks.append((ta, tb))
        core_wts.append((wa, wb))
        core_seg.append((a, b))

    # positions within (computing core -> owner) blocks; p_cap exact
    p_cap = 0
    core_pos = []
    for c in range(NCORES):
        ta, tb = core_toks[c]
        allt = np.concatenate([ta, tb])
        own = owner[allt]
        pos = np.zeros(len(allt), np.int64)
        for o in range(NCORES):
            mask = own == o
            n = int(mask.sum())
            pos[mask] = np.arange(n)
            p_cap = max(p_cap, n)
        core_pos.append(pos)
    rows = NCORES * p_cap

    nc = _build(s1, s2, p_cap)

    # host-side layouts (fp16 compute dtype)
    w1r = np.asarray(routed_w1, np.float16)              # [E, D, 2F]
    w2r = np.asarray(routed_w2, np.float16)              # [E, F, D]
    sw1r = np.asarray(shared_w1, np.float16)[0]          # [D, 2F]
    sw2r = np.asarray(shared_w2, np.float16)[0]          # [F, D]
    b1r = np.asarray(routed_b1, np.float32)
    b2r = np.asarray(routed_b2, np.float32)
    xr = x.astype(np.float16)                            # [T, D]

    def tile_w1(w):                # [D,2F] -> [mp=64, p=128, k=8, c=128]
        return np.ascontiguousarray(
            w.reshape(KD, 128, 2 * MF, 128).transpose(2, 1, 0, 3))

    def tile_w2(w):                # [F,D] -> [q=4, p=128, fj=8, d=1024]
        return np.ascontiguousarray(
            w.reshape(NSLAB, PERS, 128, D).transpose(0, 2, 1, 3))

    def col_bias(b):               # [2F] -> [128, 64]
        return np.ascontiguousarray(
            np.asarray(b, np.float32).reshape(2 * MF, 128).T)

    w1_t = {int(e): tile_w1(w1r[e]) for e in range(E)}
    w2_t = {int(e): tile_w2(w2r[e]) for e in range(E)}
    b1_t = {int(e): col_bias(b1r[e]) for e in range(E)}
    sw1_t = tile_w1(sw1r)
    sw2_t = tile_w2(sw2r)
    sb1_t = col_bias(np.asarray(shared_b1)[0])

    # a2a_out row for each (token, slot): computing core * p_cap + pos
    slot_rows = np.zeros((T, TOPK), np.int64)
    for c in range(NCORES):
        ta, tb = core_toks[c]
        allt = np.concatenate([ta, tb])
        exps = np.concatenate([np.full(len(ta), core_seg[c][0]),
                               np.full(len(tb), core_seg[c][1])])
        sel = (topk_idx[allt] == exps[:, None])
        k_of = sel[:, 1].astype(np.int64)
        slot_rows[allt, k_of] = c * p_cap + core_pos[c]

    in_maps = []
    for c in range(NCORES):
        ta, tb = core_toks[c]
        wa, wb = core_wts[c]
        ea, eb = core_seg[c]

        xg_a = np.zeros((KD, 128, c_alloc), np.float16)
        if len(ta):
            xg_a[:, :, :len(ta)] = xr[ta].T.reshape(KD, 128, len(ta))
        if len(tb):
            xg_a[:, :, A1:A1 + len(tb)] = xr[tb].T.reshape(KD, 128, len(tb))

        cw_a = np.zeros((NT * 128,), np.float32)
        scat_a = np.full((NT * 128,), 2**31 - 1, np.int32)
        sc = owner[np.concatenate([ta, tb])] * p_cap + core_pos[c]
        cw_a[:len(ta)] = wa
        cw_a[A1:A1 + len(tb)] = wb
        scat_a[:len(ta)] = sc[:len(ta)].astype(np.int32)
        scat_a[A1:A1 + len(tb)] = sc[len(ta):].astype(np.int32)
        cw_t = np.ascontiguousarray(cw_a.reshape(NT, 128).T)
        scat_t = np.ascontiguousarray(scat_a.reshape(NT, 128).T)

        xs_a = np.ascontiguousarray(
            xr[c * TO:(c + 1) * TO].T.reshape(KD, 128, TO))

        g0 = slot_rows[c * TO:(c + 1) * TO, 0].astype(np.int32)
        g1 = slot_rows[c * TO:(c + 1) * TO, 1].astype(np.int32)
        g0_t = np.ascontiguousarray(g0.reshape(TO // 128, 128).T)
        g1_t = np.ascontiguousarray(g1.reshape(TO // 128, 128).T)

        in_maps.append({
            "xg": xg_a, "xs": xs_a,
            "w1A": w1_t[ea], "w2A": w2_t[ea],
            "w1B": w1_t[eb], "w2B": w2_t[eb],
            "sw1": sw1_t, "sw2": sw2_t,
            "b1A": b1_t[ea], "b1B": b1_t[eb], "sb1": sb1_t,
            "b2A": b2r[ea].reshape(1, D).copy(),
            "b2B": b2r[eb].reshape(1, D).copy(),
            "sb2": np.asarray(shared_b2, np.float32)[0].reshape(1, D).copy(),
            "cw": cw_t, "scat": scat_t, "g0i": g0_t, "g1i": g1_t,
        })

    res = run_bass_kernel_spmd(nc, in_maps, list(range(NCORES)))
    return np.concatenate([res.results[c]["out"] for c in range(NCORES)],
                          axis=0)


# revision 16
# speedup vs baseline: 1.0995x; 1.0018x over previous
"""MoE (top-2 of 8 experts + 1 shared expert, SwiGLU FFN) on 8 TRN2 NeuronCores.

Strategy (expert-parallel with pairwise token-split load balancing):
  - Host computes the gate (softmax top-2) and the dispatch maps.
  - Experts are paired heaviest-with-lightest; each core of a pair processes
    HALF of each paired expert's tokens as two sequential FFN segments
    (seg1 = the heavy expert's half, seg2 = the light expert's half), using
    that expert's full weights. This caps the compile-time stream lengths at
    s1 = ceil(max_e c_e / 2), s2 = ceil(max-light c_e / 2) instead of the
    unsplit max c_e — less padding than plain 1-expert-per-core when expert
    loads are imbalanced, with no change to the collective structure (every
    token's contribution is computed wholly on one core).
  - Routed outputs are scaled by the gate weight and scattered into a single
    AllToAll dispatch buffer (p_cap rows per (src,dst) block, exact).
  - The AllToAll overlaps the shared-expert FFN (each core runs its own T/8
    token slice). Combine: out[t] = shared(t) + r(t), where r(t) sums the two
    routed contributions inside the indirect gather DMA (compute_op=add).
  - Host concatenates the 8 [T/8, D] output slices.

Perf notes:
  - The PE clock here is GPIO-power-capped at 13/16 x 2.4 = 1.95 GHz; the
    kernel is tensor-bound, so the levers are cycle count and keeping the
    PE fed.
  - All DMAs are batched into large single transfers (w1: one 256KB load per
    f-chunk via host retiling, w2: one 2MB load per slab, index/weight
    vectors: one load each); first loads are ordered in exact first-use order.
  - During a collective, HWDGE model DMA starves; w1 prefetch depth (bufs=8)
    rides out the single ~50us AllToAll window.
  - Stage-2 accumulators are fp16 (SBUF budget); bias adds fold into the
    stage-2 PSUM drain; final combine+store interleaves with shared stage-2.

Compute dtype fp16 (~8e-4 rel err vs fp32 reference, threshold 2e-2);
PSUM accumulation fp32.
"""
import contextlib

import numpy as np

import concourse.bass as bass
import concourse.tile as tile
from concourse import bacc, mybir
from concourse.bass_utils import run_bass_kernel_spmd

# problem shape (hardcoded per contract)
T = 8192
D = 1024
F = 4096
E = 8
TOPK = 2
NCORES = 8
TO = T // NCORES          # tokens owned per core
KD = D // 128             # 8 contraction chunks for stage 1
MF = 2 * F // 128 // 2    # 32 a-chunks (b-chunks at +MF)
NSLAB = 4
PERS = MF // NSLAB        # 8 f-chunks per slab

F32 = mybir.dt.float32
F16 = mybir.dt.float16
I32 = mybir.dt.int32

_nc_cache: dict[tuple, object] = {}


def _chunk_slices(c_len):
    """Moving-dim chunks of <=512, each >=256 where possible."""
    out = []
    pos = 0
    rem = c_len
    while rem > 0:
        if rem > 512:
            w = 512 if rem - 512 >= 256 else 384
        else:
            w = rem
        out.append((pos, w))
        pos += w
        rem -= w
    return out


def _build(s1, s2, p_cap):
    key = (s1, s2, p_cap)
    if key in _nc_cache:
        return _nc_cache[key]

    NT1 = -(-s1 // 128)
    NT2 = -(-s2 // 128)
    A1 = NT1 * 128                    # seg2's column base in xg
    NT = NT1 + NT2                    # routed token tiles total
    c_alloc = A1 + NT2 * 128
    g_alloc = max(A1, NT2 * 128, TO)
    NTS = TO // 128                   # shared token tiles (8)
    rows = NCORES * p_cap
    chunks_1 = _chunk_slices(s1)
    chunks_2 = _chunk_slices(s2)
    chunks_s = _chunk_slices(TO)

    nc = bacc.Bacc("TRN2", target_bir_lowering=False, debug=False,
                   num_devices=NCORES)

    def din(name, shape, dt):
        return nc.dram_tensor(name, shape, dt, kind="ExternalInput").ap()

    xg = din("xg", [KD, 128, c_alloc], F16)        # gathered tokens^T
    xs = din("xs", [KD, 128, TO], F16)             # owned tokens^T
    w1A = din("w1A", [2 * MF, 128, KD, 128], F16)  # [mp, p, k, c]
    w2A = din("w2A", [NSLAB, 128, PERS, D], F16)   # [q, p, fj, d]
    w1B = din("w1B", [2 * MF, 128, KD, 128], F16)
    w2B = din("w2B", [NSLAB, 128, PERS, D], F16)
    sw1 = din("sw1", [2 * MF, 128, KD, 128], F16)
    sw2 = din("sw2", [NSLAB, 128, PERS, D], F16)
    b1A = din("b1A", [128, 2 * MF], F32)           # col m = chunk-m bias
    b1B = din("b1B", [128, 2 * MF], F32)
    sb1 = din("sb1", [128, 2 * MF], F32)
    b2A = din("b2A", [1, D], F32)
    b2B = din("b2B", [1, D], F32)
    sb2 = din("sb2", [1, D], F32)
    cwd = din("cw", [128, NT], F32)                # combine weights (col t)
    scat = din("scat", [128, NT], I32)             # scatter row in a2a_in
    g0i = din("g0i", [128, NTS], I32)              # gather rows in a2a_out
    g1i = din("g1i", [128, NTS], I32)
    out = nc.dram_tensor("out", [TO, D], F32, kind="ExternalOutput").ap()

    with tile.TileContext(nc) as tc:
        with contextlib.ExitStack() as ctx:
            sbuf = ctx.enter_context(tc.tile_pool(name="sbuf", bufs=1))
            psum = ctx.enter_context(tc.tile_pool(name="psum", bufs=2,
                                                  space="PSUM"))
            dpool = ctx.enter_context(tc.tile_pool(name="dram", bufs=1,
                                                   space="DRAM"))

            a2a_in = dpool.tile([rows, D], F16)
            a2a_out = dpool.tile([rows, D], F16)

            # resident small tensors (one batched DMA each)
            b1At = sbuf.tile([128, 2 * MF], F32, tag="b1A", name="b1A",
                             bufs=1)
            b1Bt = sbuf.tile([128, 2 * MF], F32, tag="b1B", name="b1B",
                             bufs=1)
            sb1t = sbuf.tile([128, 2 * MF], F32, tag="sb1t", name="sb1t",
                             bufs=1)
            cwt = sbuf.tile([128, NT], F32, tag="cwt", name="cwt", bufs=1)
            sct = sbuf.tile([128, NT], I32, tag="sct", name="sct", bufs=1)
            g0t = sbuf.tile([128, NTS], I32, tag="g0t", name="g0t", bufs=1)
            g1t = sbuf.tile([128, NTS], I32, tag="g1t", name="g1t", bufs=1)
            nc.sync.dma_start(out=b1At[:], in_=b1A[:])
            b2At = sbuf.tile([128, D], F32, tag="b2A", name="b2A", bufs=1)
            b2Bt = sbuf.tile([128, D], F32, tag="b2B", name="b2B", bufs=1)
            sb2t = sbuf.tile([128, D], F32, tag="sb2t", name="sb2t", bufs=1)
            nc.gpsimd.dma_start(out=b2At[:], in_=b2A.to_broadcast([128, D]))
            nc.gpsimd.dma_start(out=b2Bt[:], in_=b2B.to_broadcast([128, D]))
            nc.gpsimd.dma_start(out=sb2t[:], in_=sb2.to_broadcast([128, D]))

            # DMA-queue order mirrors first-use order: seg1 fi0 weights +
            # the first stage-1 chunk's x columns land first, then the next
            # chunk's columns (chunk-major, all k) interleaved with the fi1
            # weight pair, then the rest — so early stage-1 never stalls
            c0 = chunks_1[0][1]
            w1_pre = []
            for _ in range(2):
                w1_pre.append((
                    sbuf.tile([128, KD, 128], F16, tag="w1a", name="w1a",
                              bufs=8),
                    sbuf.tile([128, KD, 128], F16, tag="w1b", name="w1b",
                              bufs=8)))
            nc.sync.dma_start(out=w1_pre[0][0][:], in_=w1A[0])
            xk = []
            for k in range(KD):
                xt = sbuf.tile([128, c_alloc], F16, tag=f"xk{k}",
                               name=f"xk{k}", bufs=1)
                nc.sync.dma_start(out=xt[:, :c0], in_=xg[k][:, :c0])
                xk.append(xt)
            nc.sync.dma_start(out=w1_pre[0][1][:], in_=w1A[MF])
            c1 = min(2 * c0, c_alloc)
            for k in range(KD):
                nc.sync.dma_start(out=xk[k][:, c0:c1], in_=xg[k][:, c0:c1])
            nc.sync.dma_start(out=w1_pre[1][0][:], in_=w1A[1])
            nc.sync.dma_start(out=w1_pre[1][1][:], in_=w1A[1 + MF])
            for k in range(KD):
                nc.sync.dma_start(out=xk[k][:, c1:], in_=xg[k][:, c1:])
            nc.sync.dma_start(out=b1Bt[:], in_=b1B[:])
            nc.sync.dma_start(out=sb1t[:], in_=sb1[:])

            g_tiles = [sbuf.tile([128, g_alloc], F16, tag=f"g{fi}",
                                 name=f"g{fi}", bufs=1)
                       for fi in range(PERS)]

            y_tiles = [sbuf.tile([128, D], F16, tag=f"y{t}", name=f"y{t}",
                                 bufs=1) for t in range(NT)]

            def g_pad(c_len, n_t):
                """Zero g pad columns so stage-2 reads finite values."""
                if c_len < n_t * 128:
                    for g_t in g_tiles:
                        nc.vector.memset(g_t[:, c_len:n_t * 128], 0.0)

            def ffn(w1d, w2d, b1t_, bias2_t, n_t, chunks, col0=0, y_off=0,
                    w1_pre=None, final=None):
                """One SwiGLU FFN pass over xk columns [col0, col0+len);
                writes y_tiles[y_off..y_off+n_t-1] (fp16, bias2 folded in).
                final=(r_tiles, out_ap) additionally emits the per-tile
                combine + output DMA inline with the last slab's stage-2."""
                for q in range(NSLAB):
                    w2t = sbuf.tile([128, PERS, D], F16, tag="w2",
                                    name="w2", bufs=2)
                    for fi in range(PERS):
                        mp = q * PERS + fi
                        if q == 0 and w1_pre is not None and fi < len(w1_pre):
                            w1a, w1b = w1_pre[fi]
                        else:
                            w1a = sbuf.tile([128, KD, 128], F16, tag="w1a",
                                            name="w1a", bufs=8)
                            w1b = sbuf.tile([128, KD, 128], F16, tag="w1b",
                                            name="w1b", bufs=8)
                            nc.sync.dma_start(out=w1a[:], in_=w1d[mp])
                            nc.sync.dma_start(out=w1b[:], in_=w1d[mp + MF])
                        if fi == 1:
                            nc.sync.dma_start(out=w2t[:], in_=w2d[q])
                        g_t = g_tiles[fi]
                        for cs, cw in chunks:
                            ps_a = psum.tile([128, 512], F32, space="PSUM",
                                             tag="ps_a", name="ps_a", bufs=3)
                            ps_b = psum.tile([128, 512], F32, space="PSUM",
                                             tag="ps_b", name="ps_b", bufs=3)
                            xs_ = slice(col0 + cs, col0 + cs + cw)
                            for k in range(KD):
                                nc.tensor.matmul(out=ps_a[:, :cw],
                                                 lhsT=w1a[:, k, :],
                                                 rhs=xk[k][:, xs_],
                                                 start=(k == 0),
                                                 stop=(k == KD - 1))
                            for k in range(KD):
                                nc.tensor.matmul(out=ps_b[:, :cw],
                                                 lhsT=w1b[:, k, :],
                                                 rhs=xk[k][:, xs_],
                                                 start=(k == 0),
                                                 stop=(k == KD - 1))
                            t_a = sbuf.tile([128, 512], F16, tag="t_a",
                                            name="t_a", bufs=2)
                            t_b = sbuf.tile([128, 512], F16, tag="t_b",
                                            name="t_b", bufs=2)
                            nc.scalar.activation(
                                t_a[:, :cw], ps_a[:, :cw],
                                mybir.ActivationFunctionType.Silu,
                                bias=b1t_[:, mp:mp + 1])
                            nc.scalar.activation(
                                t_b[:, :cw], ps_b[:, :cw],
                                mybir.ActivationFunctionType.Identity,
                                bias=b1t_[:, mp + MF:mp + MF + 1])
                            nc.vector.tensor_mul(g_t[:, cs:cs + cw],
                                                 t_a[:, :cw], t_b[:, :cw])
                    # stage-2 partial: y (+)= g_slab.T @ w2_slab
                    for t in range(n_t):
                        ts = slice(t * 128, (t + 1) * 128)
                        for dd in range(D // 512):
                            ds = slice(dd * 512, (dd + 1) * 512)
                            ps_y = psum.tile([128, 512], F32, space="PSUM",
                                             tag="ps_y", name="ps_y", bufs=2)
                            for fi in range(PERS):
                                nc.tensor.matmul(out=ps_y[:],
                                                 lhsT=g_tiles[fi][:, ts],
                                                 rhs=w2t[:, fi, ds],
                                                 start=(fi == 0),
                                                 stop=(fi == PERS - 1))
                            yt = y_tiles[y_off + t]
                            if q == 0:
                                nc.vector.tensor_add(yt[:, ds], ps_y[:],
                                                     bias2_t[:, ds])
                            else:
                                nc.vector.tensor_add(yt[:, ds], yt[:, ds],
                                                     ps_y[:])
                            if q == NSLAB - 1 and final is not None:
                                r_t, out_ap = final
                                if dd == 0:
                                    yo = sbuf.tile([128, D], F32, tag="yo",
                                                   name="yo", bufs=2)
                                nc.vector.tensor_add(yo[:, ds], yt[:, ds],
                                                     r_t[t][:, ds])
                        if q == NSLAB - 1 and final is not None:
                            nc.sync.dma_start(
                                out=final[1][t * 128:(t + 1) * 128, :],
                                in_=yo[:])

            # ---------------- routed segments (seg1 then seg2) -------------
            g_pad(s1, NT1)
            ffn(w1A, w2A, b1At, b2At, NT1, chunks_1, col0=0, y_off=0,
                w1_pre=w1_pre)
            g_pad(s2, NT2)
            ffn(w1B, w2B, b1Bt, b2Bt, NT2, chunks_2, col0=A1, y_off=NT1)
            nc.sync.dma_start(out=cwt[:], in_=cwd[:])
            nc.sync.dma_start(out=sct[:], in_=scat[:])
            nc.sync.dma_start(out=g0t[:], in_=g0i[:])
            nc.sync.dma_start(out=g1t[:], in_=g1i[:])

            # finalize: scale by combine weight, scatter into a2a_in
            for t in range(NT):
                yh = sbuf.tile([128, D], F16, tag="yh", name="yh", bufs=3)
                nc.vector.tensor_scalar_mul(yh[:], y_tiles[t][:],
                                            cwt[:, t:t + 1])
                nc.gpsimd.indirect_dma_start(
                    out=a2a_in[:],
                    out_offset=bass.IndirectOffsetOnAxis(ap=sct[:, t:t + 1],
                                                         axis=0),
                    in_=yh[:],
                    in_offset=None,
                    bounds_check=rows - 1,
                    oob_is_err=False,
                )
            nc.gpsimd.collective_compute(
                "AllToAll",
                mybir.AluOpType.bypass,
                replica_groups=[list(range(NCORES))],
                ins=[a2a_in[:].opt()],
                outs=[a2a_out[:].opt()],
            )

            # combine gathers: r[t] = contrib0 + contrib1 (accumulated in DMA)
            r_tiles = []
            for t in range(NTS):
                rt = sbuf.tile([128, D], F16, tag=f"r{t}", name=f"r{t}",
                               bufs=1)
                nc.gpsimd.indirect_dma_start(
                    out=rt[:], out_offset=None, in_=a2a_out[:],
                    in_offset=bass.IndirectOffsetOnAxis(ap=g0t[:, t:t + 1],
                                                        axis=0))
                nc.gpsimd.indirect_dma_start(
                    out=rt[:], out_offset=None, in_=a2a_out[:],
                    in_offset=bass.IndirectOffsetOnAxis(ap=g1t[:, t:t + 1],
                                                        axis=0),
                    compute_op=mybir.AluOpType.add)
                r_tiles.append(rt)

            # ---------------- shared expert on owned tokens (overlaps) -----
            for k in range(KD):
                nc.sync.dma_start(out=xk[k][:, :TO], in_=xs[k])
            ffn(sw1, sw2, sb1t, sb2t, NTS, chunks_s,
                final=(r_tiles, out))

    nc.compile()
    _nc_cache[key] = nc
    return nc


def _route(x, gate_w, gate_b):
    """Host gate: softmax top-2 (float64 for stable ordering)."""
    logits = (x.astype(np.float64) @ gate_w.astype(np.float64)
              + gate_b.astype(np.float64))
    m = logits.max(axis=-1, keepdims=True)
    p = np.exp(logits - m)
    p /= p.sum(axis=-1, keepdims=True)
    order = np.argsort(-p, axis=-1)
    idx = order[:, :TOPK]                      # [T, 2]
    wts = np.take_along_axis(p, idx, axis=-1)  # [T, 2]
    return idx, wts.astype(np.float32)


def kernel(x, gate_w, gate_b, shared_w1, shared_b1, shared_w2, shared_b2,
           routed_w1, routed_b1, routed_w2, routed_b2):
    x = np.asarray(x, dtype=np.float32)
    topk_idx, topk_w = _route(x, np.asarray(gate_w), np.asarray(gate_b))

    owner = np.arange(T) // TO                 # owning core per token

    # per-expert dispatch lists (ascending token order => owner-sorted)
    tok_lists, wt_lists = [], []
    for e in range(E):
        sel = (topk_idx == e)                  # [T, 2]
        tsel = np.nonzero(sel.any(axis=1))[0]
        k_of = sel[tsel, 1].astype(np.int64)   # slot (experts distinct)
        w_of = topk_w[tsel, :][np.arange(len(tsel)), k_of]
        tok_lists.append(tsel)
        wt_lists.append(w_of)

    counts = np.array([len(t) for t in tok_lists])
    # pair heaviest with lightest; each pair-core gets half of each expert
    order_desc = np.argsort(-counts)
    bigs = order_desc[:4]
    smalls = order_desc[4:][::-1]              # lightest first
    s1 = int(-(-counts[bigs].max() // 2))
    s2 = int(-(-counts[smalls].max() // 2))
    NT1 = -(-s1 // 128)
    NT2 = -(-s2 // 128)
    A1 = NT1 * 128
    NT = NT1 + NT2
    c_alloc = A1 + NT2 * 128

    # per-core token lists: seg1 = half of big expert, seg2 = half of small.
    # Interleaved split (even/odd positions) so each half spans all owner
    # cores evenly — a contiguous split would concentrate owners and blow
    # up the fixed per-(src,dst) AllToAll block size.
    core_exp = []                              # (expA, selA, expB, selB)
    for p in range(4):
        a, b = int(bigs[p]), int(smalls[p])
        core_exp.append((a, slice(0, None, 2), b, slice(0, None, 2)))
        core_exp.append((a, slice(1, None, 2), b, slice(1, None, 2)))

    core_toks, core_wts, core_seg = [], [], []
    for c in range(NCORES):
        a, sa, b, sb_ = core_exp[c]
        ta, tb = tok_lists[a][sa], tok_lists[b][sb_]
        wa, wb = wt_lists[a][sa], wt_lists[b][sb_]
        core_toks.append((ta, tb))
        core_wts.append((wa, wb))
        core_seg.append((a, b))

    # positions within (computing core -> owner) blocks; p_cap exact
    p_cap = 0
    core_pos = []
    for c in range(NCORES):
        ta, tb = core_toks[c]
        allt = np.concatenate([ta, tb])
        own = owner[allt]
        pos = np.zeros(len(allt), np.int64)
        for o in range(NCORES):
            mask = own == o
            n = int(mask.sum())
            pos[mask] = np.arange(n)
            p_cap = max(p_cap, n)
        core_pos.append(pos)
    rows = NCORES * p_cap

    nc = _build(s1, s2, p_cap)

    # host-side layouts (fp16 compute dtype)
    w1r = np.asarray(routed_w1, np.float16)              # [E, D, 2F]
    w2r = np.asarray(routed_w2, np.float16)              # [E, F, D]
    sw1r = np.asarray(shared_w1, np.float16)[0]          # [D, 2F]
    sw2r = np.asarray(shared_w2, np.float16)[0]          # [F, D]
    b1r = np.asarray(routed_b1, np.float32)
    b2r = np.asarray(routed_b2, np.float32)
    xr = x.astype(np.float16)                            # [T, D]

    def tile_w1(w):                # [D,2F] -> [mp=64, p=128, k=8, c=128]
        return np.ascontiguousarray(
            w.reshape(KD, 128, 2 * MF, 128).transpose(2, 1, 0, 3))

    def tile_w2(w):                # [F,D] -> [q=4, p=128, fj=8, d=1024]
        return np.ascontiguousarray(
            w.reshape(NSLAB, PERS, 128, D).transpose(0, 2, 1, 3))

    def col_bias(b):               # [2F] -> [128, 64]
        return np.ascontiguousarray(
            np.asarray(b, np.float32).reshape(2 * MF, 128).T)

    w1_t = {int(e): tile_w1(w1r[e]) for e in range(E)}
    w2_t = {int(e): tile_w2(w2r[e]) for e in range(E)}
    b1_t = {int(e): col_bias(b1r[e]) for e in range(E)}
    sw1_t = tile_w1(sw1r)
    sw2_t = tile_w2(sw2r)
    sb1_t = col_bias(np.asarray(shared_b1)[0])

    # a2a_out row for each (token, slot): computing core * p_cap + pos
    slot_rows = np.zeros((T, TOPK), np.int64)
    for c in range(NCORES):
        ta, tb = core_toks[c]
        allt = np.concatenate([ta, tb])
        exps = np.concatenate([np.full(len(ta), core_seg[c][0]),
                               np.full(len(tb), core_seg[c][1])])
        sel = (topk_idx[allt] == exps[:, None])
        k_of = sel[:, 1].astype(np.int64)
        slot_rows[allt, k_of] = c * p_cap + core_pos[c]

    in_maps = []
    for c in range(NCORES):
        ta, tb = core_toks[c]
        wa, wb = core_wts[c]
        ea, eb = core_seg[c]

        xg_a = np.zeros((KD, 128, c_alloc), np.float16)
        if len(ta):
            xg_a[:, :, :len(ta)] = xr[ta].T.reshape(KD, 128, len(ta))
        if len(tb):
            xg_a[:, :, A1:A1 + len(tb)] = xr[tb].T.reshape(KD, 128, len(tb))

        cw_a = np.zeros((NT * 128,), np.float32)
        scat_a = np.full((NT * 128,), 2**31 - 1, np.int32)
        sc = owner[np.concatenate([ta, tb])] * p_cap + core_pos[c]
        cw_a[:len(ta)] = wa
        cw_a[A1:A1 + len(tb)] = wb
        scat_a[:len(ta)] = sc[:len(ta)].astype(np.int32)
        scat_a[A1:A1 + len(tb)] = sc[len(ta):].astype(np.int32)
        cw_t = np.ascontiguousarray(cw_a.reshape(NT, 128).T)
        scat_t = np.ascontiguousarray(scat_a.reshape(NT, 128).T)

        xs_a = np.ascontiguousarray(
            xr[c * TO:(c + 1) * TO].T.reshape(KD, 128, TO))

        g0 = slot_rows[c * TO:(c + 1) * TO, 0].astype(np.int32)
        g1 = slot_rows[c * TO:(c + 1) * TO, 1].astype(np.int32)
        g0_t = np.ascontiguousarray(g0.reshape(TO // 128, 128).T)
        g1_t = np.ascontiguousarray(g1.reshape(TO // 128, 128).T)

        in_maps.append({
            "xg": xg_a, "xs": xs_a,
            "w1A": w1_t[ea], "w2A": w2_t[ea],
            "w1B": w1_t[eb], "w2B": w2_t[eb],
            "sw1": sw1_t, "sw2": sw2_t,
            "b1A": b1_t[ea], "b1B": b1_t[eb], "sb1": sb1_t,
            "b2A": b2r[ea].reshape(1, D).copy(),
            "b2B": b2r[eb].reshape(1, D).copy(),
            "sb2": np.asarray(shared_b2, np.float32)[0].reshape(1, D).copy(),
            "cw": cw_t, "scat": scat_t, "g0i": g0_t, "g1i": g1_t,
        })

    res = run_bass_kernel_spmd(nc, in_maps, list(range(NCORES)))
    return np.concatenate([res.results[c]["out"] for c in range(NCORES)],
                          axis=0)


# revision 18
# speedup vs baseline: 1.1043x; 1.0044x over previous
"""MoE (top-2 of 8 experts + 1 shared expert, SwiGLU FFN) on 8 TRN2 NeuronCores.

Strategy (expert-parallel with pairwise token-split load balancing):
  - Host computes the gate (softmax top-2) and the dispatch maps.
  - Experts are paired heaviest-with-lightest; each core of a pair processes
    HALF of each paired expert's tokens as two sequential FFN segments
    (seg1 = the heavy expert's half, seg2 = the light expert's half), using
    that expert's full weights. This caps the compile-time stream lengths at
    s1 = ceil(max_e c_e / 2), s2 = ceil(max-light c_e / 2) instead of the
    unsplit max c_e — less padding than plain 1-expert-per-core when expert
    loads are imbalanced, with no change to the collective structure (every
    token's contribution is computed wholly on one core).
  - Routed outputs are scaled by the gate weight and scattered into a single
    AllToAll dispatch buffer (p_cap rows per (src,dst) block, exact).
  - The AllToAll overlaps the shared-expert FFN (each core runs its own T/8
    token slice). Combine: out[t] = shared(t) + r(t), where r(t) sums the two
    routed contributions inside the indirect gather DMA (compute_op=add).
  - Host concatenates the 8 [T/8, D] output slices.

Perf notes:
  - The PE clock here is GPIO-power-capped at 13/16 x 2.4 = 1.95 GHz; the
    kernel is tensor-bound, so the levers are cycle count and keeping the
    PE fed.
  - All DMAs are batched into large single transfers (w1: one 256KB load per
    f-chunk via host retiling, w2: one 2MB load per slab, index/weight
    vectors: one load each); first loads are ordered in exact first-use order.
  - During a collective, HWDGE model DMA starves; w1 prefetch depth (bufs=8)
    rides out the single ~50us AllToAll window.
  - Stage-2 accumulators are fp16 (SBUF budget); bias adds fold into the
    stage-2 PSUM drain; final combine+store interleaves with shared stage-2.

Compute dtype fp16 (~8e-4 rel err vs fp32 reference, threshold 2e-2);
PSUM accumulation fp32.
"""
import contextlib

import numpy as np

import concourse.bass as bass
import concourse.tile as tile
from concourse import bacc, mybir
from concourse.bass_utils import run_bass_kernel_spmd

# problem shape (hardcoded per contract)
T = 8192
D = 1024
F = 4096
E = 8
TOPK = 2
NCORES = 8
TO = T // NCORES          # tokens owned per core
KD = D // 128             # 8 contraction chunks for stage 1
MF = 2 * F // 128 // 2    # 32 a-chunks (b-chunks at +MF)
NSLAB = 4
PERS = MF // NSLAB        # 8 f-chunks per slab

F32 = mybir.dt.float32
F16 = mybir.dt.float16
I32 = mybir.dt.int32

_nc_cache: dict[tuple, object] = {}


def _chunk_slices(c_len):
    """Moving-dim chunks of <=512, each >=256 where possible."""
    out = []
    pos = 0
    rem = c_len
    while rem > 0:
        if rem > 512:
            w = 512 if rem - 512 >= 256 else 384
        else:
            w = rem
        out.append((pos, w))
        pos += w
        rem -= w
    return out


def _build(s1, s2, p_cap):
    key = (s1, s2, p_cap)
    if key in _nc_cache:
        return _nc_cache[key]

    NT1 = -(-s1 // 128)
    NT2 = -(-s2 // 128)
    A1 = NT1 * 128                    # seg2's column base in xg
    NT = NT1 + NT2                    # routed token tiles total
    c_alloc = A1 + NT2 * 128
    g_alloc = max(A1, NT2 * 128, TO)
    NTS = TO // 128                   # shared token tiles (8)
    rows = NCORES * p_cap
    chunks_1 = _chunk_slices(s1)
    chunks_2 = _chunk_slices(s2)
    chunks_s = _chunk_slices(TO)

    nc = bacc.Bacc("TRN2", target_bir_lowering=False, debug=False,
                   num_devices=NCORES)

    def din(name, shape, dt):
        return nc.dram_tensor(name, shape, dt, kind="ExternalInput").ap()

    xg = din("xg", [KD, 128, c_alloc], F16)        # gathered tokens^T
    xs = din("xs", [KD, 128, TO], F16)             # owned tokens^T
    w1A = din("w1A", [2 * MF, 128, KD, 128], F16)  # [mp, p, k, c]
    w2A = din("w2A", [NSLAB, 128, PERS, D], F16)   # [q, p, fj, d]
    w1B = din("w1B", [2 * MF, 128, KD, 128], F16)
    w2B = din("w2B", [NSLAB, 128, PERS, D], F16)
    sw1 = din("sw1", [2 * MF, 128, KD, 128], F16)
    sw2 = din("sw2", [NSLAB, 128, PERS, D], F16)
    b1A = din("b1A", [128, 2 * MF], F32)           # col m = chunk-m bias
    b1B = din("b1B", [128, 2 * MF], F32)
    sb1 = din("sb1", [128, 2 * MF], F32)
    b2A = din("b2A", [1, D], F32)
    b2B = din("b2B", [1, D], F32)
    sb2 = din("sb2", [1, D], F32)
    cwd = din("cw", [128, NT], F32)                # combine weights (col t)
    scat = din("scat", [128, NT], I32)             # scatter row in a2a_in
    g0i = din("g0i", [128, NTS], I32)              # gather rows in a2a_out
    g1i = din("g1i", [128, NTS], I32)
    out = nc.dram_tensor("out", [TO, D], F32, kind="ExternalOutput").ap()

    with tile.TileContext(nc) as tc:
        with contextlib.ExitStack() as ctx:
            sbuf = ctx.enter_context(tc.tile_pool(name="sbuf", bufs=1))
            psum = ctx.enter_context(tc.tile_pool(name="psum", bufs=2,
                                                  space="PSUM"))
            dpool = ctx.enter_context(tc.tile_pool(name="dram", bufs=1,
                                                   space="DRAM"))

            a2a_in = dpool.tile([rows, D], F16)
            a2a_out = dpool.tile([rows, D], F16)

            # resident small tensors (one batched DMA each)
            b1At = sbuf.tile([128, 2 * MF], F32, tag="b1A", name="b1A",
                             bufs=1)
            b1Bt = sbuf.tile([128, 2 * MF], F32, tag="b1B", name="b1B",
                             bufs=1)
            sb1t = sbuf.tile([128, 2 * MF], F32, tag="sb1t", name="sb1t",
                             bufs=1)
            cwt = sbuf.tile([128, NT], F32, tag="cwt", name="cwt", bufs=1)
            sct = sbuf.tile([128, NT], I32, tag="sct", name="sct", bufs=1)
            g0t = sbuf.tile([128, NTS], I32, tag="g0t", name="g0t", bufs=1)
            g1t = sbuf.tile([128, NTS], I32, tag="g1t", name="g1t", bufs=1)
            nc.sync.dma_start(out=b1At[:], in_=b1A[:])
            b2At = sbuf.tile([128, D], F32, tag="b2A", name="b2A", bufs=1)
            b2Bt = sbuf.tile([128, D], F32, tag="b2B", name="b2B", bufs=1)
            sb2t = sbuf.tile([128, D], F32, tag="sb2t", name="sb2t", bufs=1)
            nc.gpsimd.dma_start(out=b2At[:], in_=b2A.to_broadcast([128, D]))
            nc.gpsimd.dma_start(out=b2Bt[:], in_=b2B.to_broadcast([128, D]))
            nc.gpsimd.dma_start(out=sb2t[:], in_=sb2.to_broadcast([128, D]))

            # DMA-queue order mirrors first-use order: seg1 fi0 weights +
            # the first stage-1 chunk's x columns land first, then the next
            # chunk's columns (chunk-major, all k) interleaved with the fi1
            # weight pair, then the rest — so early stage-1 never stalls
            c0 = chunks_1[0][1]
            w1_pre = []
            for _ in range(4):
                w1_pre.append((
                    sbuf.tile([128, KD, 128], F16, tag="w1a", name="w1a",
                              bufs=8),
                    sbuf.tile([128, KD, 128], F16, tag="w1b", name="w1b",
                              bufs=8)))
            nc.sync.dma_start(out=w1_pre[0][0][:], in_=w1A[0])
            xk = []
            for k in range(KD):
                xt = sbuf.tile([128, c_alloc], F16, tag=f"xk{k}",
                               name=f"xk{k}", bufs=1)
                nc.sync.dma_start(out=xt[:, :c0], in_=xg[k][:, :c0])
                xk.append(xt)
            nc.sync.dma_start(out=w1_pre[0][1][:], in_=w1A[MF])
            cuts = [c0, min(2 * c0, A1), A1,
                    (A1 + c_alloc) // 2 // 128 * 128, c_alloc]
            for ci in range(len(cuts) - 1):
                lo, hi = cuts[ci], cuts[ci + 1]
                if hi > lo:
                    for k in range(KD):
                        nc.sync.dma_start(out=xk[k][:, lo:hi],
                                          in_=xg[k][:, lo:hi])
                if ci + 1 < len(w1_pre):
                    nc.sync.dma_start(out=w1_pre[ci + 1][0][:],
                                      in_=w1A[ci + 1])
                    nc.sync.dma_start(out=w1_pre[ci + 1][1][:],
                                      in_=w1A[ci + 1 + MF])
            nc.sync.dma_start(out=b1Bt[:], in_=b1B[:])
            nc.sync.dma_start(out=sb1t[:], in_=sb1[:])

            g_tiles = [sbuf.tile([128, g_alloc], F16, tag=f"g{fi}",
                                 name=f"g{fi}", bufs=1)
                       for fi in range(PERS)]

            y_tiles = [sbuf.tile([128, D], F16, tag=f"y{t}", name=f"y{t}",
                                 bufs=1) for t in range(NT)]

            def g_pad(c_len, n_t):
                """Zero g pad columns so stage-2 reads finite values."""
                if c_len < n_t * 128:
                    for g_t in g_tiles:
                        nc.vector.memset(g_t[:, c_len:n_t * 128], 0.0)

            def ffn(w1d, w2d, b1t_, bias2_t, n_t, chunks, col0=0, y_off=0,
                    w1_pre=None, final=None):
                """One SwiGLU FFN pass over xk columns [col0, col0+len);
                writes y_tiles[y_off..y_off+n_t-1] (fp16, bias2 folded in).
                final=(r_tiles, out_ap) additionally emits the per-tile
                combine + output DMA inline with the last slab's stage-2."""
                for q in range(NSLAB):
                    w2t = sbuf.tile([128, PERS, D], F16, tag="w2",
                                    name="w2", bufs=2)
                    for fi in range(PERS):
                        mp = q * PERS + fi
                        if q == 0 and w1_pre is not None and fi < len(w1_pre):
                            w1a, w1b = w1_pre[fi]
                        else:
                            w1a = sbuf.tile([128, KD, 128], F16, tag="w1a",
                                            name="w1a", bufs=8)
                            w1b = sbuf.tile([128, KD, 128], F16, tag="w1b",
                                            name="w1b", bufs=8)
                            nc.sync.dma_start(out=w1a[:], in_=w1d[mp])
                            nc.sync.dma_start(out=w1b[:], in_=w1d[mp + MF])
                        if fi == 1:
                            nc.sync.dma_start(out=w2t[:], in_=w2d[q])
                        g_t = g_tiles[fi]
                        for cs, cw in chunks:
                            ps_a = psum.tile([128, 512], F32, space="PSUM",
                                             tag="ps_a", name="ps_a", bufs=3)
                            ps_b = psum.tile([128, 512], F32, space="PSUM",
                                             tag="ps_b", name="ps_b", bufs=3)
                            xs_ = slice(col0 + cs, col0 + cs + cw)
                            for k in range(KD):
                                nc.tensor.matmul(out=ps_a[:, :cw],
                                                 lhsT=w1a[:, k, :],
                                                 rhs=xk[k][:, xs_],
                                                 start=(k == 0),
                                                 stop=(k == KD - 1))
                            for k in range(KD):
                                nc.tensor.matmul(out=ps_b[:, :cw],
                                                 lhsT=w1b[:, k, :],
                                                 rhs=xk[k][:, xs_],
                                                 start=(k == 0),
                                                 stop=(k == KD - 1))
                            t_a = sbuf.tile([128, 512], F16, tag="t_a",
                                            name="t_a", bufs=2)
                            t_b = sbuf.tile([128, 512], F16, tag="t_b",
                                            name="t_b", bufs=2)
                            nc.scalar.activation(
                                t_a[:, :cw], ps_a[:, :cw],
                                mybir.ActivationFunctionType.Silu,
                                bias=b1t_[:, mp:mp + 1])
                            nc.scalar.activation(
                                t_b[:, :cw], ps_b[:, :cw],
                                mybir.ActivationFunctionType.Identity,
                                bias=b1t_[:, mp + MF:mp + MF + 1])
                            nc.vector.tensor_mul(g_t[:, cs:cs + cw],
                                                 t_a[:, :cw], t_b[:, :cw])
                    # stage-2 partial: y (+)= g_slab.T @ w2_slab
                    for t in range(n_t):
                        ts = slice(t * 128, (t + 1) * 128)
                        for dd in range(D // 512):
                            ds = slice(dd * 512, (dd + 1) * 512)
                            ps_y = psum.tile([128, 512], F32, space="PSUM",
                                             tag="ps_y", name="ps_y", bufs=2)
                            for fi in range(PERS):
                                nc.tensor.matmul(out=ps_y[:],
                                                 lhsT=g_tiles[fi][:, ts],
                                                 rhs=w2t[:, fi, ds],
                                                 start=(fi == 0),
                                                 stop=(fi == PERS - 1))
                            yt = y_tiles[y_off + t]
                            if q == 0:
                                nc.vector.tensor_add(yt[:, ds], ps_y[:],
                                                     bias2_t[:, ds])
                            else:
                                nc.vector.tensor_add(yt[:, ds], yt[:, ds],
                                                     ps_y[:])
                            if q == NSLAB - 1 and final is not None:
                                r_t, out_ap = final
                                if dd == 0:
                                    yo = sbuf.tile([128, D], F32, tag="yo",
                                                   name="yo", bufs=2)
                                nc.vector.tensor_add(yo[:, ds], yt[:, ds],
                                                     r_t[t][:, ds])
                                nc.sync.dma_start(
                                    out=out_ap[t * 128:(t + 1) * 128, ds],
                                    in_=yo[:, ds])

            # ---------------- routed segments (seg1 then seg2) -------------
            g_pad(s1, NT1)
            ffn(w1A, w2A, b1At, b2At, NT1, chunks_1, col0=0, y_off=0,
                w1_pre=w1_pre)
            g_pad(s2, NT2)
            ffn(w1B, w2B, b1Bt, b2Bt, NT2, chunks_2, col0=A1, y_off=NT1)
            nc.sync.dma_start(out=cwt[:], in_=cwd[:])
            nc.sync.dma_start(out=sct[:], in_=scat[:])
            nc.sync.dma_start(out=g0t[:], in_=g0i[:])
            nc.sync.dma_start(out=g1t[:], in_=g1i[:])

            # finalize: scale by combine weight, scatter into a2a_in
            for t in range(NT):
                yh = sbuf.tile([128, D], F16, tag="yh", name="yh", bufs=3)
                nc.vector.tensor_scalar_mul(yh[:], y_tiles[t][:],
                                            cwt[:, t:t + 1])
                nc.gpsimd.indirect_dma_start(
                    out=a2a_in[:],
                    out_offset=bass.IndirectOffsetOnAxis(ap=sct[:, t:t + 1],
                                                         axis=0),
                    in_=yh[:],
                    in_offset=None,
                    bounds_check=rows - 1,
                    oob_is_err=False,
                )
            nc.gpsimd.collective_compute(
                "AllToAll",
                mybir.AluOpType.bypass,
                replica_groups=[list(range(NCORES))],
                ins=[a2a_in[:].opt()],
                outs=[a2a_out[:].opt()],
            )

            # combine gathers: r[t] = contrib0 + contrib1 (accumulated in DMA)
            r_tiles = []
            for t in range(NTS):
                rt = sbuf.tile([128, D], F16, tag=f"r{t}", name=f"r{t}",
                               bufs=1)
                nc.gpsimd.indirect_dma_start(
                    out=rt[:], out_offset=None, in_=a2a_out[:],
                    in_offset=bass.IndirectOffsetOnAxis(ap=g0t[:, t:t + 1],
                                                        axis=0))
                nc.gpsimd.indirect_dma_start(
                    out=rt[:], out_offset=None, in_=a2a_out[:],
                    in_offset=bass.IndirectOffsetOnAxis(ap=g1t[:, t:t + 1],
                                                        axis=0),
                    compute_op=mybir.AluOpType.add)
                r_tiles.append(rt)

            # ---------------- shared expert on owned tokens (overlaps) -----
            for k in range(KD):
                nc.sync.dma_start(out=xk[k][:, :TO], in_=xs[k])
            ffn(sw1, sw2, sb1t, sb2t, NTS, chunks_s,
                final=(r_tiles, out))

    nc.compile()
    _nc_cache[key] = nc
    return nc


def _route(x, gate_w, gate_b):
    """Host gate: softmax top-2 (float64 for stable ordering)."""
    logits = (x.astype(np.float64) @ gate_w.astype(np.float64)
              + gate_b.astype(np.float64))
    m = logits.max(axis=-1, keepdims=True)
    p = np.exp(logits - m)
    p /= p.sum(axis=-1, keepdims=True)
    order = np.argsort(-p, axis=-1)
    idx = order[:, :TOPK]                      # [T, 2]
    wts = np.take_along_axis(p, idx, axis=-1)  # [T, 2]
    return idx, wts.astype(np.float32)


def kernel(x, gate_w, gate_b, shared_w1, shared_b1, shared_w2, shared_b2,
           routed_w1, routed_b1, routed_w2, routed_b2):
    x = np.asarray(x, dtype=np.float32)
    topk_idx, topk_w = _route(x, np.asarray(gate_w), np.asarray(gate_b))

    owner = np.arange(T) // TO                 # owning core per token

    # per-expert dispatch lists (ascending token order => owner-sorted)
    tok_lists, wt_lists = [], []
    for e in range(E):
        sel = (topk_idx == e)                  # [T, 2]
        tsel = np.nonzero(sel.any(axis=1))[0]
        k_of = sel[tsel, 1].astype(np.int64)   # slot (experts distinct)
        w_of = topk_w[tsel, :][np.arange(len(tsel)), k_of]
        tok_lists.append(tsel)
        wt_lists.append(w_of)

    counts = np.array([len(t) for t in tok_lists])
    # pair heaviest with lightest; each pair-core gets half of each expert
    order_desc = np.argsort(-counts)
    bigs = order_desc[:4]
    smalls = order_desc[4:][::-1]              # lightest first
    s1 = int(-(-counts[bigs].max() // 2))
    s2 = int(-(-counts[smalls].max() // 2))
    NT1 = -(-s1 // 128)
    NT2 = -(-s2 // 128)
    A1 = NT1 * 128
    NT = NT1 + NT2
    c_alloc = A1 + NT2 * 128

    # per-core token lists: seg1 = half of big expert, seg2 = half of small.
    # Interleaved split (even/odd positions) so each half spans all owner
    # cores evenly — a contiguous split would concentrate owners and blow
    # up the fixed per-(src,dst) AllToAll block size.
    core_exp = []                              # (expA, selA, expB, selB)
    for p in range(4):
        a, b = int(bigs[p]), int(smalls[p])
        core_exp.append((a, slice(0, None, 2), b, slice(0, None, 2)))
        core_exp.append((a, slice(1, None, 2), b, slice(1, None, 2)))

    core_toks, core_wts, core_seg = [], [], []
    for c in range(NCORES):
        a, sa, b, sb_ = core_exp[c]
        ta, tb = tok_lists[a][sa], tok_lists[b][sb_]
        wa, wb = wt_lists[a][sa], wt_lists[b][sb_]
        core_toks.append((ta, tb))
        core_wts.append((wa, wb))
        core_seg.append((a, b))

    # positions within (computing core -> owner) blocks; p_cap exact
    p_cap = 0
    core_pos = []
    for c in range(NCORES):
        ta, tb = core_toks[c]
        allt = np.concatenate([ta, tb])
        own = owner[allt]
        pos = np.zeros(len(allt), np.int64)
        for o in range(NCORES):
            mask = own == o
            n = int(mask.sum())
            pos[mask] = np.arange(n)
            p_cap = max(p_cap, n)
        core_pos.append(pos)
    rows = NCORES * p_cap

    nc = _build(s1, s2, p_cap)

    # host-side layouts (fp16 compute dtype)
    w1r = np.asarray(routed_w1, np.float16)              # [E, D, 2F]
    w2r = np.asarray(routed_w2, np.float16)              # [E, F, D]
    sw1r = np.asarray(shared_w1, np.float16)[0]          # [D, 2F]
    sw2r = np.asarray(shared_w2, np.float16)[0]          # [F, D]
    b1r = np.asarray(routed_b1, np.float32)
    b2r = np.asarray(routed_b2, np.float32)
    xr = x.astype(np.float16)                            # [T, D]

    def tile_w1(w):                # [D,2F] -> [mp=64, p=128, k=8, c=128]
        return np.ascontiguousarray(
            w.reshape(KD, 128, 2 * MF, 128).transpose(2, 1, 0, 3))

    def tile_w2(w):                # [F,D] -> [q=4, p=128, fj=8, d=1024]
        return np.ascontiguousarray(
            w.reshape(NSLAB, PERS, 128, D).transpose(0, 2, 1, 3))

    def col_bias(b):               # [2F] -> [128, 64]
        return np.ascontiguousarray(
            np.asarray(b, np.float32).reshape(2 * MF, 128).T)

    w1_t = {int(e): tile_w1(w1r[e]) for e in range(E)}
    w2_t = {int(e): tile_w2(w2r[e]) for e in range(E)}
    b1_t = {int(e): col_bias(b1r[e]) for e in range(E)}
    sw1_t = tile_w1(sw1r)
    sw2_t = tile_w2(sw2r)
    sb1_t = col_bias(np.asarray(shared_b1)[0])

    # a2a_out row for each (token, slot): computing core * p_cap + pos
    slot_rows = np.zeros((T, TOPK), np.int64)
    for c in range(NCORES):
        ta, tb = core_toks[c]
        allt = np.concatenate([ta, tb])
        exps = np.concatenate([np.full(len(ta), core_seg[c][0]),
                               np.full(len(tb), core_seg[c][1])])
        sel = (topk_idx[allt] == exps[:, None])
        k_of = sel[:, 1].astype(np.int64)
        slot_rows[allt, k_of] = c * p_cap + core_pos[c]

    in_maps = []
    for c in range(NCORES):
        ta, tb = core_toks[c]
        wa, wb = core_wts[c]
        ea, eb = core_seg[c]

        xg_a = np.zeros((KD, 128, c_alloc), np.float16)
        if len(ta):
            xg_a[:, :, :len(ta)] = xr[ta].T.reshape(KD, 128, len(ta))
        if len(tb):
            xg_a[:, :, A1:A1 + len(tb)] = xr[tb].T.reshape(KD, 128, len(tb))

        cw_a = np.zeros((NT * 128,), np.float32)
        scat_a = np.full((NT * 128,), 2**31 - 1, np.int32)
        sc = owner[np.concatenate([ta, tb])] * p_cap + core_pos[c]
        cw_a[:len(ta)] = wa
        cw_a[A1:A1 + len(tb)] = wb
        scat_a[:len(ta)] = sc[:len(ta)].astype(np.int32)
        scat_a[A1:A1 + len(tb)] = sc[len(ta):].astype(np.int32)
        cw_t = np.ascontiguousarray(cw_a.reshape(NT, 128).T)
        scat_t = np.ascontiguousarray(scat_a.reshape(NT, 128).T)

        xs_a = np.ascontiguousarray(
            xr[c * TO:(c + 1) * TO].T.reshape(KD, 128, TO))

        g0 = slot_rows[c * TO:(c + 1) * TO, 0].astype(np.int32)
        g1 = slot_rows[c * TO:(c + 1) * TO, 1].astype(np.int32)
        g0_t = np.ascontiguousarray(g0.reshape(TO // 128, 128).T)
        g1_t = np.ascontiguousarray(g1.reshape(TO // 128, 128).T)

        in_maps.append({
            "xg": xg_a, "xs": xs_a,
            "w1A": w1_t[ea], "w2A": w2_t[ea],
            "w1B": w1_t[eb], "w2B": w2_t[eb],
            "sw1": sw1_t, "sw2": sw2_t,
            "b1A": b1_t[ea], "b1B": b1_t[eb], "sb1": sb1_t,
            "b2A": b2r[ea].reshape(1, D).copy(),
            "b2B": b2r[eb].reshape(1, D).copy(),
            "sb2": np.asarray(shared_b2, np.float32)[0].reshape(1, D).copy(),
            "cw": cw_t, "scat": scat_t, "g0i": g0_t, "g1i": g1_t,
        })

    res = run_bass_kernel_spmd(nc, in_maps, list(range(NCORES)))
    return np.concatenate([res.results[c]["out"] for c in range(NCORES)],
                          axis=0)
